# revision 1
# baseline (speedup 1.0000x reference)
"""Trainium2 Bass kernel for a pre-LN transformer block (MHA + FFN).

Data-parallel over batch: 8 NeuronCores, one batch element each.
All matmuls run as float32r (full PE rate at free-dim>=256), storage fp32.
"""
import sys

for _p in ("/opt/trn_rl_repo", "/root/.axon_site/_ro/trn_rl_repo"):
    if _p not in sys.path:
        sys.path.insert(0, _p)

import numpy as np
import concourse.bass as bass
import concourse.tile as tile
from concourse import bacc, mybir
from concourse.bass import ds, ts
from concourse.bass_utils import run_bass_kernel_spmd
from concourse.masks import make_identity

P = 128
N = 1024          # tokens per core (seq len)
D = 1024          # d_emb
H = 16            # heads
HS = 64           # head size
FF = 4096         # ffn hidden
NT = N // P       # 8 token tiles
DB = D // P       # 8 d blocks
EBS = D // P      # 8 e blocks (qkv out features)
NH = 2            # n halves of 512
LN_EPS = 1e-5

F32 = mybir.dt.float32
R = mybir.dt.float32r
AF = mybir.ActivationFunctionType
OP = mybir.AluOpType

_CACHED_NC = None


def build_nc(use_lrelu=True):
    nc = bacc.Bacc("TRN2", target_bir_lowering=False, debug=False, num_devices=8)

    x_d = nc.dram_tensor("x", [N, D], F32, kind="ExternalInput").ap()
    wq_d = nc.dram_tensor("Wq", [H, D, HS], F32, kind="ExternalInput").ap()
    bq_d = nc.dram_tensor("bq", [H, HS], F32, kind="ExternalInput").ap()
    wk_d = nc.dram_tensor("Wk", [H, D, HS], F32, kind="ExternalInput").ap()
    bk_d = nc.dram_tensor("bk", [H, HS], F32, kind="ExternalInput").ap()
    wv_d = nc.dram_tensor("Wv", [H, D, HS], F32, kind="ExternalInput").ap()
    bv_d = nc.dram_tensor("bv", [H, HS], F32, kind="ExternalInput").ap()
    wp_d = nc.dram_tensor("Wproj", [H * HS, D], F32, kind="ExternalInput").ap()
    bp_d = nc.dram_tensor("bproj", [D], F32, kind="ExternalInput").ap()
    w1_d = nc.dram_tensor("W1", [D, FF], F32, kind="ExternalInput").ap()
    b1_d = nc.dram_tensor("b1", [FF], F32, kind="ExternalInput").ap()
    w2_d = nc.dram_tensor("W2", [FF, D], F32, kind="ExternalInput").ap()
    b2_d = nc.dram_tensor("b2", [D], F32, kind="ExternalInput").ap()
    g1_d = nc.dram_tensor("ln1_g", [D], F32, kind="ExternalInput").ap()
    c1_d = nc.dram_tensor("ln1_b", [D], F32, kind="ExternalInput").ap()
    g2_d = nc.dram_tensor("ln2_g", [D], F32, kind="ExternalInput").ap()
    c2_d = nc.dram_tensor("ln2_b", [D], F32, kind="ExternalInput").ap()
    out_d = nc.dram_tensor("out", [N, D], F32, kind="ExternalOutput").ap()
    x2pb_d = nc.dram_tensor("x2pb_scratch", [P, NT, D], F32).ap()

    with tile.TileContext(nc) as tc:
        with tc.tile_pool(name="cn", bufs=1) as cp, \
             tc.tile_pool(name="big", bufs=1) as bp:
            # ---- constants / bias vectors (persistent, tiny) ----
            ident = cp.tile([P, P], F32)
            make_identity(nc, ident[:])
            ones_f = cp.tile([P, 1], F32)
            nc.vector.memset(ones_f[:], 1.0)
            ones64 = cp.tile([1, HS], R)
            nc.vector.tensor_copy(ones64[:],
                                  ones_f[0:1, :].to_broadcast([1, HS]))
            onesP = cp.tile([1, P], R)
            nc.vector.tensor_copy(onesP[:],
                                  ones_f[0:1, :].to_broadcast([1, P]))
            epsv = cp.tile([P, 1], F32)
            nc.vector.memset(epsv[:], LN_EPS)
            identR = cp.tile([P, P], R)
            nc.vector.tensor_copy(identR[:], ident[:])

            # x load first so the big DMA isn't stuck behind the
            # scattered little bias loads
            xsb = bp.tile([P, NT, D], F32, tag="at", name="xsb")
            xr3 = x_d.rearrange("(t p) d -> p t d", p=P)
            for tb in range(NT):
                nc.sync.dma_start(xsb[:, tb, :], xr3[:, tb, :])

            bqv = cp.tile([P, EBS], F32)
            nc.sync.dma_start(bqv[:], bq_d.rearrange("h s -> (h s)")
                              .rearrange("(b p) -> p b", p=P))
            bkv = cp.tile([P, EBS], F32)
            nc.sync.dma_start(bkv[:], bk_d.rearrange("h s -> (h s)")
                              .rearrange("(b p) -> p b", p=P))
            bvv = cp.tile([P, EBS], F32)
            nc.sync.dma_start(bvv[:], bv_d.rearrange("h s -> (h s)")
                              .rearrange("(b p) -> p b", p=P))
            g1v = cp.tile([P, DB], F32)
            nc.sync.dma_start(g1v[:], g1_d.rearrange("(b p) -> p b", p=P))
            c1v = cp.tile([P, DB], F32)
            nc.sync.dma_start(c1v[:], c1_d.rearrange("(b p) -> p b", p=P))
            g2v = cp.tile([P, DB], F32)
            nc.sync.dma_start(g2v[:], g2_d.rearrange("(b p) -> p b", p=P))
            c2v = cp.tile([P, DB], F32)
            nc.sync.dma_start(c2v[:], c2_d.rearrange("(b p) -> p b", p=P))
            b1v = cp.tile([P, FF // P], F32)
            nc.sync.dma_start(b1v[:], b1_d.rearrange("(b p) -> p b", p=P))

            # LN stats scratch (reused for LN2 by tag)
            st_sum = cp.tile([P, NT], F32)
            st_sq = cp.tile([P, NT], F32)
            st_mu = cp.tile([P, NT], F32)
            st_var = cp.tile([P, NT], F32)
            st_rs = cp.tile([P, NT], F32)
            st_nm = cp.tile([P, NT], F32)
            st_vh = cp.tile([P, NT], F32)
            st_t = cp.tile([P, NT], F32)
            st_ih = cp.tile([P, NT], mybir.dt.int32)

            def layernorm_transpose(src, dst, gv, cv, pfx, pspool, trbufs,
                                    after_tb=None, tbs=None):
                """src: [P, NT, D] token layout (f32) -> dst [P, DB, N] f32r
                feature layout, with affine (gv, cv per-partition) folded into
                the transpose evacuation. Fully per-tb so each token tile
                flows stats -> normalize -> transpose independently."""
                for tb in (range(NT) if tbs is None else tbs):
                    t1 = (tb, tb + 1)
                    nc.vector.reduce_sum(st_sum[:, t1[0]:t1[1]], src[:, tb, :],
                                         axis=mybir.AxisListType.X)
                    sq = bp.tile([P, D], F32, tag="qb", bufs=2,
                                 name=f"sq{tb}")
                    nc.scalar.activation(sq[:], src[:, tb, :], AF.Square,
                                         accum_out=st_sq[:, t1[0]:t1[1]])
                    sm = st_sum[:, t1[0]:t1[1]]
                    var = st_var[:, t1[0]:t1[1]]
                    rs = st_rs[:, t1[0]:t1[1]]
                    nm = st_nm[:, t1[0]:t1[1]]
                    ih = st_ih[:, t1[0]:t1[1]]
                    vh = st_vh[:, t1[0]:t1[1]]
                    tt = st_t[:, t1[0]:t1[1]]
                    i32 = mybir.dt.int32
                    # var = sq/D - (sum/D)^2 + eps   (depth-3 chain)
                    nc.vector.tensor_tensor(tt, sm, sm, OP.mult)
                    nc.vector.tensor_scalar(var, tt, -1.0 / (D * D), LN_EPS,
                                            OP.mult, OP.add)
                    nc.vector.tensor_scalar(var, st_sq[:, t1[0]:t1[1]],
                                            1.0 / D, var, OP.mult, OP.add)
                    # rstd = rsqrt(var), DVE-only (bit hack + 2 Newton steps)
                    # so the ACT engine never needs the sqrt table set
                    nc.vector.tensor_scalar(ih, var.bitcast(i32), 1, None,
                                            OP.arith_shift_right)
                    nc.vector.tensor_scalar(rs.bitcast(i32), ih, -1,
                                            0x5F3759DF, OP.mult, OP.add)
                    nc.vector.tensor_scalar_mul(vh, var, -0.5)
                    for _ in range(2):
                        nc.vector.tensor_tensor(tt, rs, rs, OP.mult)
                        nc.vector.tensor_scalar(tt, tt, vh, 1.5,
                                                OP.mult, OP.add)
                        nc.vector.tensor_tensor(rs, rs, tt, OP.mult)
                    # nm = -(sum/D)*rstd
                    nc.vector.tensor_tensor(nm, sm, rs, OP.mult)
                    nc.vector.tensor_scalar_mul(nm, nm, -1.0 / D)
                    tnorm = bp.tile([P, D], F32, tag="kb", bufs=2,
                                    name=f"tn{tb}")
                    nc.vector.tensor_scalar(tnorm[:], src[:, tb, :],
                                            rs, nm, OP.mult, OP.add)
                    for db in range(DB):
                        pt = pspool.tile([P, P], F32, tag="tr", bufs=trbufs,
                                         name=f"ptr{pfx}_{tb}_{db}")
                        nc.tensor.transpose(pt[:], tnorm[:, ts(db, P)],
                                            ident[:])
                        nc.vector.tensor_scalar(dst[:, db, ts(tb, P)], pt[:],
                                                gv[:, db:db + 1],
                                                cv[:, db:db + 1],
                                                OP.mult, OP.add)
                    if after_tb is not None:
                        after_tb(tb)

            # ================= Phase A: LN1 + transpose ====================
            HT = bp.tile([P, DB, N], R, tag="ht", name="HT")
            psAB_cm = tc.tile_pool(name="psAB", bufs=1, space="PSUM")
            psAB = psAB_cm.__enter__()
            layernorm_transpose(xsb, HT, g1v, c1v, "a", psAB, 4)

            # ================= Phase B0: V projection ======================
            Vaug = bp.tile([P, NT, H, HS + 1], R, tag="v", name="Vaug")
            nc.vector.tensor_copy(
                Vaug[:, :, :, HS:HS + 1],
                ones_f[:, None, :].to_broadcast([P, NT, H, 1]))
            if True:
                for eh in range(2):
                    wvt = bp.tile([P, DB, 512], R, tag="se", bufs=2,
                                  name=f"wv{eh}")
                    for do in range(DB):
                        nc.sync.dma_start(
                            wvt[:, do].rearrange("p (h s) -> p h s", s=HS),
                            wv_d[eh * 8:(eh + 1) * 8, ds(do * P, P), :]
                            .rearrange("h dp s -> dp h s")
                            .bitcast(R))
                    for tb in range(NT):
                        pv = psAB.tile([P, 512], F32, tag="qkv", bufs=4,
                                       name=f"pv{eh}_{tb}")
                        for db in range(DB):
                            nc.tensor.matmul(pv[:], HT[:, db, ts(tb, P)],
                                             wvt[:, db, :],
                                             start=(db == 0), stop=(db == DB - 1))
                        nc.scalar.activation(
                            Vaug[:, tb, eh * 8:(eh + 1) * 8, 0:HS],
                            pv[:].rearrange("p (h s) -> p h s", s=HS), AF.Copy)

            # ============ Phase BC: Q/K per e-block fused with attention ===
            psAB_cm.__exit__(None, None, None)
            psBC_cm = tc.tile_pool(name="psBC", bufs=1, space="PSUM")
            psBC = psBC_cm.__enter__()
            attnT = bp.tile([P, EBS, N], R, tag="at", name="attnT")
            # prefetch Wproj during attention (4 x 1MB quarter-tiles)
            wp4 = []
            for g4 in range(4):
                wpt = bp.tile([P, 2, D], R, tag="sh", bufs=4, name=f"wp{g4}")
                nc.sync.dma_start(
                    wpt[:], wp_d[ds(g4 * 256, 256)]
                    .rearrange("(eo ep) d -> ep eo d", ep=P)
                    .bitcast(R))
                wp4.append(wpt)

            if True:
                for eb in range(EBS):
                    wqt = bp.tile([P, DB, P], R, tag="wqk", bufs=2,
                                  name=f"wq{eb}")
                    for do in range(DB):
                        nc.sync.dma_start(
                            wqt[:, do].rearrange("p (h s) -> p h s", s=HS),
                            wq_d[2 * eb:2 * eb + 2, ds(do * P, P), :]
                            .rearrange("h dp s -> dp h s")
                            .bitcast(R))
                    wkt = bp.tile([P, DB, P], R, tag="wqk", bufs=2,
                                  name=f"wk{eb}")
                    for do in range(DB):
                        nc.sync.dma_start(
                            wkt[:, do].rearrange("p (h s) -> p h s", s=HS),
                            wk_d[2 * eb:2 * eb + 2, ds(do * P, P), :]
                            .rearrange("h dp s -> dp h s")
                            .bitcast(R))
                    Qb = bp.tile([P, N], R, tag="qb", bufs=2, name=f"Qb{eb}")
                    Kb = bp.tile([P, N], R, tag="kb", bufs=2, name=f"Kb{eb}")
                    for nh in range(NH):
                        pq = psBC.tile([P, 512], F32, tag="qk", bufs=2,
                                       name=f"pq{eb}_{nh}")
                        for db in range(DB):
                            nc.tensor.matmul(pq[:], wqt[:, db, :],
                                             HT[:, db, ds(nh * 512, 512)],
                                             start=(db == 0), stop=(db == DB - 1))
                        nc.vector.tensor_scalar_add(Qb[:, ds(nh * 512, 512)],
                                                    pq[:], bqv[:, eb:eb + 1])
                        pk = psBC.tile([P, 512], F32, tag="qk", bufs=2,
                                       name=f"pk{eb}_{nh}")
                        for db in range(DB):
                            nc.tensor.matmul(pk[:], wkt[:, db, :],
                                             HT[:, db, ds(nh * 512, 512)],
                                             start=(db == 0), stop=(db == DB - 1))
                        nc.vector.tensor_scalar_add(Kb[:, ds(nh * 512, 512)],
                                                    pk[:], bkv[:, eb:eb + 1])

                    # attention for heads 2eb (partitions 0:64) and
                    # 2eb+1 (partitions 64:128), per n-half of 512
                    for nh in range(NH):
                        pts = [bp.tile([P, NT, 512], R, tag="se", bufs=2,
                                       name=f"PT{eb}_{nh}_{i}")
                               for i in range(2)]
                        # scoresT[m, n] = sum_s K[m,s] Q[n,s]; exp via ACT
                        for mt in range(NT):
                            for i in range(2):
                                base = i * HS
                                pss = psBC.tile([P, 512], F32, tag="sc",
                                                bufs=4, name=f"ps{eb}{nh}{mt}{i}")
                                nc.tensor.matmul(
                                    pss[:],
                                    Kb[base:base + HS, ts(mt, P)],
                                    Qb[base:base + HS, ds(nh * 512, 512)],
                                    start=True, stop=True)
                                nc.scalar.activation(pts[i][:, mt, :], pss[:],
                                                     AF.Exp, scale=0.125)
                        pas = [psBC.tile([HS + 1, 512], F32, tag="at65",
                                         bufs=2, name=f"pa{eb}_{nh}_{i}")
                               for i in range(2)]
                        for mb in range(NT):
                            for i in range(2):
                                nc.tensor.matmul(pas[i][:],
                                                 Vaug[:, mb, 2 * eb + i, :],
                                                 pts[i][:, mb, :],
                                                 start=(mb == 0),
                                                 stop=(mb == NT - 1))
                        for i in range(2):
                            base = i * HS
                            rec = bp.tile([1, 512], F32, tag="rb", bufs=2,
                                          name=f"rc{eb}_{nh}_{i}")
                            nc.vector.reciprocal(rec[:],
                                                 pas[i][HS:HS + 1, :])
                            rbs = bp.tile([HS, 512], F32, tag="rb", bufs=2,
                                          name=f"rb{eb}_{nh}_{i}")
                            nc.gpsimd.partition_broadcast(rbs[:], rec[:])
                            dstA = attnT[base:base + HS, eb,
                                         ds(nh * 512, 512)]
                            nc.vector.tensor_tensor(dstA, pas[i][0:HS, :],
                                                    rbs[:], OP.mult)
                            nc.vector.tensor_scalar_add(
                                dstA, dstA, bvv[base:base + HS, eb:eb + 1])

            # w1(ft0) prefetch into "se" (frees at end of attention);
            # high priority so the DMA issues as soon as the slot frees
            w1pre = bp.tile([P, DB, 512], R, tag="se", bufs=2,
                            name="w1pre")
            with tc.high_priority():
                nc.sync.dma_start(
                    w1pre[:],
                    w1_d[:, ds(0, 512)]
                    .rearrange("(do dp) f -> dp do f", dp=P)
                    .bitcast(R))

            # ================= Phase D: proj + residual ====================
            psBC_cm.__exit__(None, None, None)
            psDE_cm = tc.tile_pool(name="psDE", bufs=1, space="PSUM")
            psDE = psDE_cm.__enter__()
            x2 = bp.tile([P, NT, D], F32, tag="v", name="x2")
            xr = bp.tile([P, NT, D], R, tag="ht", name="xrl")
            if True:
                # broadcast bproj -> [P, D]
                bprow = bp.tile([1, D], R, tag="kb", bufs=2, name="bprow")
                nc.sync.dma_start(bprow[:], bp_d[None, :].bitcast(R))
                bpB = bp.tile([P, D], F32, tag="qb", bufs=2, name="bpB")
                for dh in range(2):
                    pbb = psDE.tile([P, 512], F32, tag="trb", bufs=1,
                                    name=f"pbb{dh}")
                    nc.tensor.matmul(pbb[:], onesP[:],
                                     bprow[:, ds(dh * 512, 512)],
                                     start=True, stop=True)
                    nc.vector.tensor_copy(bpB[:, ds(dh * 512, 512)], pbb[:])
                for tb in range(NT):
                    nc.sync.dma_start(xr[:, tb, :],
                                      xr3[:, tb, :].bitcast(R))
                    nc.vector.tensor_tensor(xr[:, tb, :], xr[:, tb, :],
                                            bpB[:], OP.add)
                for tb in range(NT):
                    for dt in range(2):
                        pp = psDE.tile([P, 512], F32, tag="pj", bufs=4,
                                       name=f"pp{tb}_{dt}")
                        for g4 in range(4):
                            for eo in range(2):
                                nc.tensor.matmul(
                                    pp[:], attnT[:, g4 * 2 + eo, ts(tb, P)],
                                    wp4[g4][:, eo, ds(dt * 512, 512)],
                                    start=(g4 == 0 and eo == 0),
                                    stop=False)
                        # residual folded into the PE accumulation
                        nc.tensor.matmul(pp[:], identR[:],
                                         xr[:, tb, ds(dt * 512, 512)],
                                         start=False, stop=True)
                        nc.scalar.activation(x2[:, tb, ds(dt * 512, 512)],
                                             pp[:], AF.Copy)

            # ================= Phase E: LN2, transpose, stash x2+b2 ========
            H2T = bp.tile([P, DB, N], R, tag="ht", name="H2T")
            if True:
                def ffn1_group(nt, ft, fc, _unused, pool, ptag, pbufs,
                               ydst, w1t, w1o):
                    p1 = pool.tile([P, 512], F32, tag=ptag, bufs=pbufs,
                                   name=f"p1_{nt}_{ft}_{fc}")
                    for db in range(DB):
                        nc.tensor.matmul(
                            p1[:], w1t[:, db, ds(w1o, P)],
                            H2T[:, db, ds(nt * 512, 512)],
                            start=(db == 0), stop=(db == DB - 1))
                    bf = ft * 4 + fc
                    if use_lrelu:
                        nc.scalar.activation(ydst, p1[:], AF.Prelu,
                                             bias=b1v[:, bf:bf + 1],
                                             alpha=0.01)
                    else:
                        z = bp.tile([P, 512], F32, tag="qb", bufs=2,
                                    name=f"z{nt}_{bf}")
                        nc.scalar.activation(z[:], p1[:], AF.Identity,
                                             bias=b1v[:, bf:bf + 1])
                        zs = bp.tile([P, 512], F32, tag="rb", bufs=2,
                                     name=f"zs{nt}_{bf}")
                        nc.vector.tensor_scalar_mul(zs[:], z[:], 0.01)
                        nc.vector.tensor_tensor(ydst, z[:], zs[:], OP.max)

                layernorm_transpose(x2, H2T, g2v, c2v, "e", psDE, 3)
                # broadcast b2 -> [P, D]; x2 += b2B; stash to DRAM
                b2row = bp.tile([1, D], R, tag="kb", bufs=2, name="b2row")
                nc.sync.dma_start(b2row[:], b2_d[None, :].bitcast(R))
                b2B = bp.tile([P, D], F32, tag="qb", bufs=2, name="b2B")
                for dh in range(2):
                    pb2 = psDE.tile([P, 512], F32, tag="trb", bufs=1,
                                    name=f"pb2{dh}")
                    nc.tensor.matmul(pb2[:], onesP[:],
                                     b2row[:, ds(dh * 512, 512)],
                                     start=True, stop=True)
                    nc.vector.tensor_copy(b2B[:, ds(dh * 512, 512)], pb2[:])
                for tb in range(NT):
                    nc.vector.tensor_tensor(x2[:, tb, :], x2[:, tb, :],
                                            b2B[:], OP.add)
                    nc.sync.dma_start(x2pb_d[:, tb, :], x2[:, tb, :])

            # ================= Phase F: FFN ================================
            psDE_cm.__exit__(None, None, None)
            psF_cm = tc.tile_pool(name="psF", bufs=1, space="PSUM")
            psF = psF_cm.__enter__()
            if True:
                for nt in range(NH):
                    y1 = [bp.tile([P, 16, 512], R, tag=tg,
                                  name=f"y1{nt}{tg}")
                          for tg in ("at", "v")]
                    xcf = bp.tile([P, 4, D], F32, tag="se", bufs=2,
                                  name=f"xcf{nt}")
                    nc.sync.dma_start(xcf[:], x2pb_d[:, nt * 4:(nt + 1) * 4, :])
                    for ft in range(FF // 512):
                        if ft == 0:
                            w1h = [w1pre, w1pre]
                            w1off = [0, 256]
                        else:
                            w1h = []
                            w1off = [0, 0]
                            for hh in range(2):
                                w1t = bp.tile([P, DB, 256], R, tag="sh",
                                              bufs=4, name=f"w1_{nt}_{ft}_{hh}")
                                nc.sync.dma_start(
                                    w1t[:],
                                    w1_d[:, ds(ft * 512 + hh * 256, 256)]
                                    .rearrange("(do dp) f -> dp do f", dp=P)
                                    .bitcast(R))
                                w1h.append(w1t)
                        for fc in range(4):
                            bf = ft * 4 + fc
                            ffn1_group(nt, ft, fc, None, psF, "fp", 8,
                                       y1[bf // 16][:, bf % 16, :],
                                       w1h[fc // 2],
                                       w1off[fc // 2] + (fc % 2) * P)
                    pf2 = [psF.tile([P, 512], F32, tag="fp", bufs=8,
                                    name=f"p2_{nt}_{j}") for j in range(8)]
                    NFT = FF // 512
                    def w2_halves(nt, ft):
                        hs = []
                        for hh in range(2):
                            w2t = bp.tile([P, 2, D], R, tag="sh", bufs=4,
                                          name=f"w2_{nt}_{ft}_{hh}")
                            nc.sync.dma_start(
                                w2t[:],
                                w2_d[ds(ft * 512 + hh * 256, 256), :]
                                .rearrange("(fo fp) d -> fp fo d", fp=P)
                                .bitcast(R))
                            hs.append(w2t)
                        return hs
                    for ft in range(NFT - 1):
                        w2h = w2_halves(nt, ft)
                        for fc in range(4):
                            bf = ft * 4 + fc
                            ysrc = y1[bf // 16][:, bf % 16, :]
                            for tb in range(4):
                                for dt in range(2):
                                    nc.tensor.matmul(
                                        pf2[tb * 2 + dt][:],
                                        ysrc[:, ts(tb, P)],
                                        w2h[fc // 2][:, fc % 2,
                                                     ds(dt * 512, 512)],
                                        start=(ft == 0 and fc == 0),
                                        stop=False)
                    # last f-tile: close each psum group in turn so its evac
                    # and output DMA overlap the remaining groups' matmuls
                    ftl = NFT - 1
                    w2h = w2_halves(nt, ftl)
                    for tb in range(4):
                        for dt in range(2):
                            for fc in range(4):
                                bf = ftl * 4 + fc
                                ysrc = y1[bf // 16][:, bf % 16, :]
                                nc.tensor.matmul(
                                    pf2[tb * 2 + dt][:],
                                    ysrc[:, ts(tb, P)],
                                    w2h[fc // 2][:, fc % 2,
                                                 ds(dt * 512, 512)],
                                    start=False, stop=(fc == 3))
                            rows = ds(nt * 512 + tb * P, P)
                            og = bp.tile([P, 512], F32, tag="rb", bufs=2,
                                         name=f"og{nt}_{tb}_{dt}")
                            nc.vector.tensor_tensor(og[:], pf2[tb * 2 + dt][:],
                                                    xcf[:, tb, ds(dt * 512, 512)],
                                                    OP.add)
                            nc.sync.dma_start(out_d[rows, ds(dt * 512, 512)],
                                              og[:])
            psF_cm.__exit__(None, None, None)
    nc.compile()
    return nc


def get_nc():
    global _CACHED_NC
    if _CACHED_NC is None:
        _CACHED_NC = build_nc()
    return _CACHED_NC


def kernel(**inputs):
    nc = get_nc()
    x = np.ascontiguousarray(np.asarray(inputs["x"], dtype=np.float32))
    B = x.shape[0]
    weights = {k: np.ascontiguousarray(np.asarray(v, dtype=np.float32))
               for k, v in inputs.items() if k != "x"}
    in_maps = [dict(weights, x=x[b]) for b in range(B)]
    res = run_bass_kernel_spmd(nc, in_maps, list(range(B)))
    return np.stack([res.results[b]["out"] for b in range(B)], axis=0)



# revision 38
# speedup vs baseline: 1.2620x; 1.2620x over previous
"""Trainium2 Bass kernel for a pre-LN transformer block (MHA + FFN).

Data-parallel over batch: 8 NeuronCores, one batch element each.
FFN runs as fp8(e4m3) DoubleRow matmuls with full error correction:
   y@W ~= y8@W8 + y8@dW8 + dy8@W8
(each chain at 0.5 cyc/row over k-pairs = 0.75x the f32r cycle count,
residual noise ~0.1%). LN2's affine is folded into W1/b1 on the host.
Attention matmuls stay float32r in this revision.
"""
import sys

for _p in ("/opt/trn_rl_repo", "/root/.axon_site/_ro/trn_rl_repo"):
    if _p not in sys.path:
        sys.path.insert(0, _p)

import numpy as np
import ml_dtypes
import concourse.bass as bass
import concourse.tile as tile
from concourse import bacc, mybir
from concourse.bass import ds, ts
from concourse.bass_utils import run_bass_kernel_spmd
from concourse.masks import make_identity

E4M3 = (ml_dtypes.float8_e4m3fn if hasattr(ml_dtypes, "float8_e4m3fn")
        else ml_dtypes.float8_e4m3)
SW1 = 32.0      # host scale for W1 fp8
SWP = 32.0      # host scale for Wproj fp8
SW2 = 64.0      # host scale for W2 fp8

P = 128
N = 1024          # tokens per core (seq len)
D = 1024          # d_emb
H = 16            # heads
HS = 64           # head size
FF = 4096         # ffn hidden
NT = N // P       # 8 token tiles
DB = D // P       # 8 d blocks
EBS = D // P      # 8 e blocks (qkv out features)
NH = 2            # n halves of 512
LN_EPS = 1e-5

F32 = mybir.dt.float32
R = mybir.dt.float32r
F8 = mybir.dt.float8e4
U8 = mybir.dt.uint8
AF = mybir.ActivationFunctionType
OP = mybir.AluOpType
DRM = mybir.MatmulPerfMode.DoubleRow
FO = FF // P      # 32 f-blocks for FFN2 contraction

_CACHED_NC = None


def build_nc(use_lrelu=True):
    nc = bacc.Bacc("TRN2", target_bir_lowering=False, debug=False, num_devices=8)

    x_d = nc.dram_tensor("x", [N, D], F32, kind="ExternalInput").ap()
    wq8_d = nc.dram_tensor("Wq8", [P, DB, H * HS], U8, kind="ExternalInput").ap()
    bq_d = nc.dram_tensor("bq", [H, HS], F32, kind="ExternalInput").ap()
    wk8_d = nc.dram_tensor("Wk8", [P, DB, H * HS], U8, kind="ExternalInput").ap()
    bk_d = nc.dram_tensor("bk", [H, HS], F32, kind="ExternalInput").ap()
    wv8_d = nc.dram_tensor("Wv8", [P, DB, H * HS], U8, kind="ExternalInput").ap()
    bv_d = nc.dram_tensor("bv", [H, HS], F32, kind="ExternalInput").ap()
    wp8_d = nc.dram_tensor("Wp8", [P, EBS, D], mybir.dt.uint8, kind="ExternalInput").ap()
    bp_d = nc.dram_tensor("bproj", [D], F32, kind="ExternalInput").ap()
    w1c_d = nc.dram_tensor("W1c", [FF // 256, P, DB, 2, 256], U8,
                           kind="ExternalInput").ap()
    b1_d = nc.dram_tensor("b1f", [FF], F32, kind="ExternalInput").ap()
    w2c_d = nc.dram_tensor("W2c", [FO // 2, P, 2, 2, D], U8,
                           kind="ExternalInput").ap()
    b2_d = nc.dram_tensor("b2", [D], F32, kind="ExternalInput").ap()
    out_d = nc.dram_tensor("out", [N, D], F32, kind="ExternalOutput").ap()
    x2pb_d = nc.dram_tensor("x2pb_scratch", [P, NT, D], F32).ap()

    with tile.TileContext(nc) as tc:
        with tc.tile_pool(name="cn", bufs=1) as cp, \
             tc.tile_pool(name="big", bufs=1) as bp:
            # ---- constants / bias vectors (persistent, tiny) ----
            ident = cp.tile([P, P], F32)
            make_identity(nc, ident[:])
            ones_f = cp.tile([P, 1], F32)
            nc.vector.memset(ones_f[:], 1.0)
            ones64 = cp.tile([1, HS], R)
            nc.vector.tensor_copy(ones64[:],
                                  ones_f[0:1, :].to_broadcast([1, HS]))
            onesP = cp.tile([1, P], R)
            nc.vector.tensor_copy(onesP[:],
                                  ones_f[0:1, :].to_broadcast([1, P]))
            epsv = cp.tile([P, 1], F32)
            nc.vector.memset(epsv[:], LN_EPS)
            identR = cp.tile([P, P], R)
            nc.vector.tensor_copy(identR[:], ident[:])
            identR64 = cp.tile([P, P], R)
            nc.vector.tensor_scalar_mul(identR64[:], ident[:], 64.0)
            onesP64 = cp.tile([1, P], R)
            nc.vector.tensor_scalar_mul(onesP64[:], onesP[:], 64.0)
            identR32 = cp.tile([P, P], R)
            nc.vector.tensor_scalar_mul(identR32[:], ident[:], 32.0)
            onesP32 = cp.tile([1, P], R)
            nc.vector.tensor_scalar_mul(onesP32[:], onesP[:], 32.0)
            b2row = cp.tile([1, D], R)
            nc.sync.dma_start(b2row[:], b2_d[None, :].bitcast(R))

            # x load first so the big DMA isn't stuck behind the
            # scattered little bias loads
            xsb = bp.tile([P, NT, D], R, tag="at", name="xsb")
            xr3 = x_d.rearrange("(t p) d -> p t d", p=P)
            for tb in range(NT):
                nc.sync.dma_start(xsb[:, tb, :], xr3[:, tb, :].bitcast(R))

            # biases arrive pre-folded (LN affine) and pre-scaled (x32)
            bqv = cp.tile([P, EBS], F32)
            nc.sync.dma_start(bqv[:], bq_d.rearrange("h s -> (h s)")
                              .rearrange("(b p) -> p b", p=P))
            bkv = cp.tile([P, EBS], F32)
            nc.sync.dma_start(bkv[:], bk_d.rearrange("h s -> (h s)")
                              .rearrange("(b p) -> p b", p=P))
            bvrow = cp.tile([1, H * HS], R)
            nc.sync.dma_start(bvrow[:], bv_d.rearrange("h s -> (h s)")
                              [None, :].bitcast(R))
            b1v = cp.tile([P, FF // P], F32)
            nc.sync.dma_start(b1v[:], b1_d.rearrange("(b p) -> p b", p=P))
            bneg2 = cp.tile([P, 1], F32)
            nc.vector.memset(bneg2[:], -2.0)
            wq8t = bp.tile([P, DB, H * HS], F8, tag="wq8", name="wq8t")
            nc.sync.dma_start(wq8t[:], wq8_d.bitcast(F8))
            wk8t = bp.tile([P, DB, H * HS], F8, tag="wk8", name="wk8t")
            nc.sync.dma_start(wk8t[:], wk8_d.bitcast(F8))
            wv8t = bp.tile([P, DB, H * HS], F8, tag="wv8", name="wv8t")
            nc.sync.dma_start(wv8t[:], wv8_d.bitcast(F8))

            # LN stats scratch (reused for LN2 by tag)
            st_sum = cp.tile([P, NT], F32)
            st_sq = cp.tile([P, NT], F32)
            st_var = cp.tile([P, NT], F32)
            st_rs = cp.tile([P, NT], F32)
            st_nm = cp.tile([P, NT], F32)
            st_vh = cp.tile([P, NT], F32)
            st_t = cp.tile([P, NT], F32)
            st_ih = cp.tile([P, NT], mybir.dt.int32)

            def layernorm_transpose(src, dst, pfx, pspool, trbufs,
                                    delta=False):
                """src: [P, NT, D] token layout (f32) -> dst [P, DB, N] fp8
                feature layout (normalized, no affine -- folded into the
                consumer weights on the host). With delta=True, dst is
                [P, DB, 2, N]: slot 0 = fp8 value, slot 1 = fp8 residual.
                Stats run as one batched 8-wide chain over all token tiles.
                """
                def stats_group(g0, g1):
                    for tb in range(g0, g1):
                        nc.vector.reduce_sum(st_sum[:, tb:tb + 1],
                                             src[:, tb, :],
                                             axis=mybir.AxisListType.X)
                        sq = bp.tile([P, D], F32, tag="qb", bufs=2,
                                     name=f"sq{pfx}")
                        nc.scalar.activation(sq[:], src[:, tb, :], AF.Square,
                                             accum_out=st_sq[:, tb:tb + 1])
                    sm = st_sum[:, g0:g1]
                    var = st_var[:, g0:g1]
                    rs = st_rs[:, g0:g1]
                    nm = st_nm[:, g0:g1]
                    ih = st_ih[:, g0:g1]
                    vh = st_vh[:, g0:g1]
                    tt = st_t[:, g0:g1]
                    i32 = mybir.dt.int32
                    # var = sq/D - (sum/D)^2 + eps   (depth-3 chain)
                    nc.vector.tensor_tensor(tt, sm, sm, OP.mult)
                    nc.vector.tensor_scalar(var, tt, -1.0 / (D * D), LN_EPS,
                                            OP.mult, OP.add)
                    nc.vector.tensor_scalar_mul(tt, st_sq[:, g0:g1], 1.0 / D)
                    nc.vector.tensor_tensor(var, tt, var, OP.add)
                    # rstd = rsqrt(var), DVE-only (bit hack + 2 Newton steps)
                    # so the ACT engine never needs the sqrt table set
                    nc.vector.tensor_scalar(ih, var.bitcast(i32), 1, None,
                                            OP.arith_shift_right)
                    nc.vector.tensor_scalar(rs.bitcast(i32), ih, -1,
                                            0x5F3759DF, OP.mult, OP.add)
                    nc.vector.tensor_scalar_mul(vh, var, -0.5)
                    for _ in range(2):
                        nc.vector.tensor_tensor(tt, rs, rs, OP.mult)
                        nc.vector.tensor_tensor(tt, tt, vh, OP.mult)
                        nc.vector.tensor_scalar(tt, tt, 1.0, 1.5,
                                                OP.mult, OP.add)
                        nc.vector.tensor_tensor(rs, rs, tt, OP.mult)
                    # nm = -(sum/D)*rstd
                    nc.vector.tensor_tensor(nm, sm, rs, OP.mult)
                    nc.vector.tensor_scalar_mul(nm, nm, -1.0 / D)

                for (a, b) in ((0, 1), (1, 2), (2, 4), (4, 8)):
                    stats_group(a, b)
                for tb in range(NT):
                    tnorm = bp.tile([P, D], F32, tag="kb", bufs=2,
                                    name=f"tn{pfx}")
                    nc.gpsimd.tensor_scalar(tnorm[:], src[:, tb, :],
                                             st_rs[:, tb:tb + 1],
                                             st_nm[:, tb:tb + 1],
                                             OP.mult, OP.add)
                    for dh in range(2):
                        pt4 = pspool.tile([P, 4, 512], F32, tag="tr",
                                          bufs=trbufs,
                                          name=f"ptr{pfx}_{tb}_{dh}")
                        for j in range(4):
                            nc.tensor.transpose(pt4[:, j, 0:P],
                                                tnorm[:, ts(dh * 4 + j, P)],
                                                ident[:])
                        d0 = dh * 4
                        if delta:
                            nc.scalar.activation(
                                dst[:, d0:d0 + 4, 0, ts(tb, P)],
                                pt4[:, :, 0:P], AF.Copy)
                            nc.vector.tensor_tensor(
                                dst[:, d0:d0 + 4, 1, ts(tb, P)],
                                pt4[:, :, 0:P],
                                dst[:, d0:d0 + 4, 0, ts(tb, P)],
                                OP.subtract)
                        else:
                            nc.vector.tensor_copy(
                                dst[:, d0:d0 + 4, ts(tb, P)],
                                pt4[:, :, 0:P])

            # ================= Phase A: LN1 + transpose ====================
            HT = bp.tile([P, DB, N], F8, tag="h8", name="HT")
            psAB_cm = tc.tile_pool(name="psAB", bufs=1, space="PSUM")
            psAB = psAB_cm.__enter__()
            layernorm_transpose(xsb, HT, "a", psAB, 1)

            # ================= Phase B0: V projection ======================
            Vaug = bp.tile([P, NT, H, HS + 1], F8, tag="v8", name="Vaug")
            nc.vector.tensor_copy(
                Vaug[:, :, :, HS:HS + 1],
                ones_f[:, None, :].to_broadcast([P, NT, H, 1]))
            if True:
                for eh in range(2):
                    for tb in range(NT):
                        pv = psAB.tile([P, 512], F32, tag="qkv", bufs=4,
                                       name=f"pv{eh}_{tb}")
                        for kp in range(DB // 2):
                            nc.tensor.matmul(
                                pv[:], HT[:, 2 * kp:2 * kp + 2, ts(tb, P)],
                                wv8t[:, 2 * kp:2 * kp + 2,
                                     ds(eh * 512, 512)],
                                start=(kp == 0), stop=False, perf_mode=DRM)
                        # += 32*bv' via ones row (psum carries 32x values)
                        nc.tensor.matmul(pv[:], onesP[:],
                                         bvrow[:, ds(eh * 512, 512)],
                                         start=False, stop=True)
                        nc.scalar.activation(
                            Vaug[:, tb, eh * 8:(eh + 1) * 8, 0:HS],
                            pv[:].rearrange("p (h s) -> p h s", s=HS), AF.Copy,
                            scale=1.0 / 32.0)

            # ============ Phase BC: Q/K per e-block fused with attention ===
            psAB_cm.__exit__(None, None, None)
            psBC_cm = tc.tile_pool(name="psBC", bufs=1, space="PSUM")
            psBC = psBC_cm.__enter__()
            attnT8 = bp.tile([P, EBS, N], F8, tag="a8", name="attnT8")
            # prefetch Wproj (fp8) during attention
            wp8 = bp.tile([P, EBS, D], F8, tag="sh", bufs=1, name="wp8")
            nc.sync.dma_start(wp8[:], wp8_d.bitcast(F8))

            if True:
                for eb in range(EBS):
                    Qb = bp.tile([P, N], R, tag="qb", bufs=2, name=f"Qb{eb}")
                    Kb = bp.tile([P, N], R, tag="kb", bufs=2, name=f"Kb{eb}")
                    ecs = ds(eb * P, P)
                    for nh in range(NH):
                        pq = psBC.tile([P, 512], F32, tag="qk", bufs=2,
                                       name=f"pq{eb}_{nh}")
                        for kp in range(DB // 2):
                            nc.tensor.matmul(pq[:],
                                             wq8t[:, 2 * kp:2 * kp + 2, ecs],
                                             HT[:, 2 * kp:2 * kp + 2,
                                                ds(nh * 512, 512)],
                                             start=(kp == 0), stop=(kp == 3),
                                             perf_mode=DRM)
                        nc.vector.tensor_scalar_add(Qb[:, ds(nh * 512, 512)],
                                                    pq[:], bqv[:, eb:eb + 1])
                        pk = psBC.tile([P, 512], F32, tag="qk", bufs=2,
                                       name=f"pk{eb}_{nh}")
                        for kp in range(DB // 2):
                            nc.tensor.matmul(pk[:],
                                             wk8t[:, 2 * kp:2 * kp + 2, ecs],
                                             HT[:, 2 * kp:2 * kp + 2,
                                                ds(nh * 512, 512)],
                                             start=(kp == 0), stop=(kp == 3),
                                             perf_mode=DRM)
                        nc.vector.tensor_scalar_add(Kb[:, ds(nh * 512, 512)],
                                                    pk[:], bkv[:, eb:eb + 1])

                    # attention for heads 2eb (partitions 0:64) and
                    # 2eb+1 (partitions 64:128), per n-half of 512
                    for nh in range(NH):
                        pts = [bp.tile([P, NT, 512], F8, tag="p8", bufs=2,
                                       name=f"PT{eb}_{nh}_{i}")
                               for i in range(2)]
                        # scoresT[m, n] = sum_s K[m,s] Q[n,s]; exp via ACT
                        # (Qb/Kb carry 32x values -> scale 0.125/1024); the
                        # -2 bias keeps exp within fp8 range, cancels in the
                        # softmax ratio
                        for mq in range(NT // 2):
                            for i in range(2):
                                base = i * HS
                                pss = psBC.tile([P, 2, 512], F32, tag="sc",
                                                bufs=2,
                                                name=f"ps{eb}{nh}{mq}{i}")
                                for mj in range(2):
                                    nc.tensor.matmul(
                                        pss[:, mj, :],
                                        Kb[base:base + HS,
                                           ts(mq * 2 + mj, P)],
                                        Qb[base:base + HS,
                                           ds(nh * 512, 512)],
                                        start=True, stop=True)
                                nc.scalar.activation(
                                    pts[i][:, 2 * mq:2 * mq + 2, :], pss[:],
                                    AF.Exp, scale=0.125 / 1024.0,
                                    bias=bneg2[:])
                        pas = [psBC.tile([HS + 1, 512], F32, tag="at65",
                                         bufs=2, name=f"pa{eb}_{nh}_{i}")
                               for i in range(2)]
                        for mp in range(NT // 2):
                            for i in range(2):
                                nc.tensor.matmul(pas[i][:],
                                                 Vaug[:, 2 * mp:2 * mp + 2,
                                                      2 * eb + i, :],
                                                 pts[i][:, 2 * mp:2 * mp + 2,
                                                        :],
                                                 start=(mp == 0),
                                                 stop=(mp == NT // 2 - 1),
                                                 perf_mode=DRM)
                        for i in range(2):
                            base = i * HS
                            rec = bp.tile([1, 512], F32, tag="rb", bufs=4,
                                          name=f"rc{eb}_{nh}_{i}")
                            nc.vector.reciprocal(rec[:],
                                                 pas[i][HS:HS + 1, :])
                            rbs = bp.tile([HS, 512], F32, tag="rb", bufs=4,
                                          name=f"rb{eb}_{nh}_{i}")
                            nc.gpsimd.partition_broadcast(rbs[:], rec[:])
                            dstA = attnT8[base:base + HS, eb,
                                          ds(nh * 512, 512)]
                            nc.vector.tensor_tensor(dstA, pas[i][0:HS, :],
                                                    rbs[:], OP.mult)

            # w1(ft0) prefetch into "se" (frees at end of attention);
            # high priority so the DMA issues as soon as the slot frees
            w1pre = bp.tile([P, DB, 2, 256], F8, tag="w1s", bufs=3,
                            name="w1pre")
            with tc.high_priority():
                nc.gpsimd.dma_start(w1pre[:], w1c_d[0].bitcast(F8))

            # ================= Phase D: proj + residual ====================
            psBC_cm.__exit__(None, None, None)
            psDE_cm = tc.tile_pool(name="psDE", bufs=1, space="PSUM")
            psDE = psDE_cm.__enter__()
            x2 = bp.tile([P, NT, D], R, tag="ht", name="x2")
            if True:
                bprow = bp.tile([1, D], R, tag="kb", bufs=2, name="bprow")
                nc.sync.dma_start(bprow[:], bp_d[None, :].bitcast(R))
                for tb in range(NT):
                    for dt in range(2):
                        pp = psDE.tile([P, 512], F32, tag="pj", bufs=4,
                                       name=f"pp{tb}_{dt}")
                        for gp in range(4):
                            nc.tensor.matmul(
                                pp[:],
                                attnT8[:, 2 * gp:2 * gp + 2, ts(tb, P)],
                                wp8[:, 2 * gp:2 * gp + 2, ds(dt * 512, 512)],
                                start=(gp == 0), stop=False, perf_mode=DRM)
                        # bias + residual folded into the PE accumulation
                        # (psum carries 32x the true values)
                        nc.tensor.matmul(pp[:], onesP32[:],
                                         bprow[:, ds(dt * 512, 512)],
                                         start=False, stop=False)
                        nc.tensor.matmul(pp[:], identR32[:],
                                         xsb[:, tb, ds(dt * 512, 512)],
                                         start=False, stop=True)
                        nc.scalar.activation(x2[:, tb, ds(dt * 512, 512)],
                                             pp[:], AF.Copy,
                                             scale=1.0 / 32.0)

            # ============ Phase E: LN2 (affine folded into W1/b1 on host),
            # transpose into fp8 pair (h28, dh28); stash x2+b2 ==============
            H2x = bp.tile([P, DB, 2, N], F8, tag="se", bufs=1, name="H2x")
            if True:
                layernorm_transpose(x2, H2x, "e", psDE, 1, delta=True)

            # ================= Phase F: FFN (fp8 DR, 3-chain corrected) ====
            psDE_cm.__exit__(None, None, None)
            psF_cm = tc.tile_pool(name="psF", bufs=1, space="PSUM")
            psF = psF_cm.__enter__()
            if True:
                def ffn1_group(nt, ft, fc, pool, ptag, pbufs, w1t):
                    p1 = pool.tile([P, 512], F32, tag=ptag, bufs=pbufs,
                                   name=f"p1_{nt}_{ft}_{fc}")
                    fcs = ds(fc * P, P)
                    n12 = 0
                    for ch in range(3):
                        wsl, hsl = ((0, 0), (1, 0), (0, 1))[ch]
                        for kp in range(DB // 2):
                            nc.tensor.matmul(
                                p1[:],
                                w1t[:, 2 * kp:2 * kp + 2, wsl, fcs],
                                H2x[:, 2 * kp:2 * kp + 2, hsl,
                                    ds(nt * 512, 512)],
                                start=(n12 == 0), stop=(n12 == 11),
                                perf_mode=DRM)
                            n12 += 1
                    bf = ft * 2 + fc
                    if use_lrelu:
                        mz = bp.tile([P, 512], F32, tag="rb", bufs=4,
                                     name=f"mz{nt}_{bf}")
                        nc.scalar.activation(mz[:], p1[:], AF.Prelu,
                                             bias=b1v[:, bf:bf + 1],
                                             scale=1.0 / SW1,
                                             alpha=0.01)
                    else:
                        z = bp.tile([P, 512], F32, tag="qb", bufs=2,
                                    name=f"z{nt}_{bf}")
                        nc.scalar.activation(z[:], p1[:], AF.Identity,
                                             bias=b1v[:, bf:bf + 1],
                                             scale=1.0 / SW1)
                        zs = bp.tile([P, 512], F32, tag="rb", bufs=4,
                                     name=f"zs{nt}_{bf}")
                        nc.vector.tensor_scalar_mul(zs[:], z[:], 0.01)
                        mz = bp.tile([P, 512], F32, tag="rb", bufs=4,
                                     name=f"mz{nt}_{bf}")
                        nc.vector.tensor_tensor(mz[:], z[:], zs[:], OP.max)
                    # quantize on Pool; residual sub on DVE (Pool was the
                    # FFN1 pacing engine)
                    nc.vector.tensor_copy(zx[:, bf, 0, :], mz[:])
                    nc.vector.tensor_tensor(zx[:, bf, 1, :], mz[:],
                                            zx[:, bf, 0, :], OP.subtract)

                for nt in range(NH):
                    zx = bp.tile([P, FO, 2, 512], F8, tag="at",
                                 name=f"zx{nt}")
                    for ft in range(FF // 256):
                        if ft == 0 and nt == 0:
                            w1t = w1pre
                        else:
                            w1t = bp.tile([P, DB, 2, 256], F8, tag="w1s",
                                          bufs=3, name=f"w1_{nt}_{ft}")
                            nc.gpsimd.dma_start(w1t[:],
                                                w1c_d[ft].bitcast(F8))
                        for fc in range(2):
                            ffn1_group(nt, ft, fc, psF, "fp", 8, w1t)
                    pf2 = [psF.tile([P, 512], F32, tag="fp", bufs=8,
                                    name=f"p2_{nt}_{j}") for j in range(8)]
                    NG = FO // 2   # 16 fo-pairs

                    def w2_pair(nt, g):
                        w2t = bp.tile([P, 2, 2, D], F8, tag="w2s", bufs=3,
                                      name=f"w2_{nt}_{g}")
                        nc.gpsimd.dma_start(w2t[:], w2c_d[g].bitcast(F8))
                        return w2t

                    def ffn2_chains(g, w2t, tb, dt, start):
                        for ch in range(3):
                            wsl, zsl = ((0, 0), (1, 0), (0, 1))[ch]
                            nc.tensor.matmul(
                                pf2[tb * 2 + dt][:],
                                zx[:, 2 * g:2 * g + 2, zsl, ts(tb, P)],
                                w2t[:, :, wsl, ds(dt * 512, 512)],
                                start=(start and ch == 0), stop=False,
                                perf_mode=DRM)

                    for g in range(NG - 1):
                        w2t = w2_pair(nt, g)
                        for tb in range(4):
                            for dt in range(2):
                                ffn2_chains(g, w2t, tb, dt, g == 0)
                    # last fo-pair: close each psum group in turn; fold the
                    # residual (64*(x2+b2)) via identity matmul, evac 1/64
                    gl = NG - 1
                    w2t = w2_pair(nt, gl)
                    for tb in range(4):
                        for dt in range(2):
                            ffn2_chains(gl, w2t, tb, dt, False)
                            nc.tensor.matmul(
                                pf2[tb * 2 + dt][:], onesP64[:],
                                b2row[:, ds(dt * 512, 512)],
                                start=False, stop=False)
                            nc.tensor.matmul(
                                pf2[tb * 2 + dt][:], identR64[:],
                                x2[:, nt * 4 + tb, ds(dt * 512, 512)],
                                start=False, stop=True)
                            rows = ds(nt * 512 + tb * P, P)
                            og = bp.tile([P, 512], F32, tag="rb", bufs=4,
                                         name=f"og{nt}_{tb}_{dt}")
                            nc.scalar.activation(og[:], pf2[tb * 2 + dt][:],
                                                 AF.Copy, scale=1.0 / 64.0)
                            nc.sync.dma_start(out_d[rows, ds(dt * 512, 512)],
                                              og[:])
            psF_cm.__exit__(None, None, None)
    nc.compile()
    return nc


def get_nc():
    global _CACHED_NC
    if _CACHED_NC is None:
        _CACHED_NC = build_nc()
    return _CACHED_NC


def _q8pair(w, s):
    """fp8(s*w) and same-scale fp8 residual, as uint8 bit views."""
    hi = (s * w).astype(E4M3)
    lo = (s * w - hi.astype(np.float32)).astype(E4M3)
    return (np.ascontiguousarray(hi).view(np.uint8),
            np.ascontiguousarray(lo).view(np.uint8))


def prep_weights(inputs):
    f32 = lambda k: np.asarray(inputs[k], dtype=np.float64)
    g2, c2 = f32("ln2_g"), f32("ln2_b")
    W1, b1, W2 = f32("W1"), f32("b1"), f32("W2")
    # fold LN2 affine into W1/b1
    W1f = W1 * g2[:, None]
    b1f = (b1 + c2 @ W1).astype(np.float32)
    w1l = W1f.reshape(DB, P, FF).transpose(1, 0, 2).astype(np.float32)
    w2l = W2.reshape(FO, P, D).transpose(1, 0, 2).astype(np.float32)
    W18, dW18 = _q8pair(w1l, SW1)   # [P, DB, FF] uint8 views
    W28, dW28 = _q8pair(w2l, SW2)   # [P, FO, D]
    # pack (W, dW) pairs into per-tile contiguous blobs
    w1s = np.stack([W18, dW18], axis=2)            # [P, DB, 2, FF]
    W1c = np.ascontiguousarray(
        w1s.reshape(P, DB, 2, FF // 256, 256)
        .transpose(3, 0, 1, 2, 4))                 # [16, P, DB, 2, 256]
    w2s_ = np.stack([W28, dW28], axis=2)           # [P, FO, 2, D]
    W2c = np.ascontiguousarray(
        w2s_.reshape(P, FO // 2, 2, 2, D)
        .transpose(1, 0, 2, 3, 4))                 # [16, P, 2, 2, D]
    Wp = f32("Wproj")
    Wp8 = np.ascontiguousarray(
        (SWP * Wp.reshape(EBS, P, D).transpose(1, 0, 2))
        .astype(E4M3)).view(np.uint8)
    g1, c1 = f32("ln1_g"), f32("ln1_b")

    def qkvfold(wname, bname):
        W, b = f32(wname), f32(bname)          # [H, D, HS], [H, HS]
        Wf = W * g1[None, :, None]
        bf = b + np.einsum('d,hds->hs', c1, W)
        wl = (Wf.transpose(1, 0, 2).reshape(DB, P, H * HS)
              .transpose(1, 0, 2).astype(np.float32))
        w8 = np.ascontiguousarray((SWP * wl).astype(E4M3)).view(np.uint8)
        return w8, np.ascontiguousarray((SWP * bf).astype(np.float32))

    Wq8, bq32 = qkvfold("Wq", "bq")
    Wk8, bk32 = qkvfold("Wk", "bk")
    Wv8, bv32 = qkvfold("Wv", "bv")
    w = {k: np.ascontiguousarray(np.asarray(inputs[k], dtype=np.float32))
         for k in ("bproj", "b2")}
    w.update(W1c=W1c, b1f=b1f, W2c=W2c, Wp8=Wp8,
             Wq8=Wq8, bq=bq32, Wk8=Wk8, bk=bk32, Wv8=Wv8, bv=bv32)
    return w


def kernel(**inputs):
    nc = get_nc()
    x = np.ascontiguousarray(np.asarray(inputs["x"], dtype=np.float32))
    B = x.shape[0]
    weights = prep_weights(inputs)
    in_maps = [dict(weights, x=x[b]) for b in range(B)]
    res = run_bass_kernel_spmd(nc, in_maps, list(range(B)))
    return np.stack([res.results[b]["out"] for b in range(B)], axis=0)



# revision 44
# speedup vs baseline: 1.2965x; 1.0274x over previous
"""Trainium2 Bass kernel for a pre-LN transformer block (MHA + FFN).

Data-parallel over batch: 8 NeuronCores, one batch element each.
FFN runs as fp8(e4m3) DoubleRow matmuls with full error correction:
   y@W ~= y8@W8 + y8@dW8 + dy8@W8
(each chain at 0.5 cyc/row over k-pairs = 0.75x the f32r cycle count,
residual noise ~0.1%). LN2's affine is folded into W1/b1 on the host.
Attention matmuls stay float32r in this revision.
"""
import sys

for _p in ("/opt/trn_rl_repo", "/root/.axon_site/_ro/trn_rl_repo"):
    if _p not in sys.path:
        sys.path.insert(0, _p)

import numpy as np
import ml_dtypes
import concourse.bass as bass
import concourse.tile as tile
from concourse import bacc, mybir
from concourse.bass import ds, ts
from concourse.bass_utils import run_bass_kernel_spmd
from concourse.masks import make_identity

E4M3 = (ml_dtypes.float8_e4m3fn if hasattr(ml_dtypes, "float8_e4m3fn")
        else ml_dtypes.float8_e4m3)
SW1 = 32.0      # host scale for W1 fp8
SWP = 32.0      # host scale for Wproj fp8
SW2 = 64.0      # host scale for W2 fp8

P = 128
N = 1024          # tokens per core (seq len)
D = 1024          # d_emb
H = 16            # heads
HS = 64           # head size
FF = 4096         # ffn hidden
NT = N // P       # 8 token tiles
DB = D // P       # 8 d blocks
EBS = D // P      # 8 e blocks (qkv out features)
NH = 2            # n halves of 512
LN_EPS = 1e-5

F32 = mybir.dt.float32
R = mybir.dt.float32r
F8 = mybir.dt.float8e4
U8 = mybir.dt.uint8
AF = mybir.ActivationFunctionType
OP = mybir.AluOpType
DRM = mybir.MatmulPerfMode.DoubleRow
FO = FF // P      # 32 f-blocks for FFN2 contraction

_CACHED_NC = None


def build_nc(use_lrelu=True):
    nc = bacc.Bacc("TRN2", target_bir_lowering=False, debug=False, num_devices=8)

    x_d = nc.dram_tensor("x", [N, D], F32, kind="ExternalInput").ap()
    wq8_d = nc.dram_tensor("Wq8", [P, DB, H * HS], U8, kind="ExternalInput").ap()
    bq_d = nc.dram_tensor("bq", [H, HS], F32, kind="ExternalInput").ap()
    wk8_d = nc.dram_tensor("Wk8", [P, DB, H * HS], U8, kind="ExternalInput").ap()
    bk_d = nc.dram_tensor("bk", [H, HS], F32, kind="ExternalInput").ap()
    wv8_d = nc.dram_tensor("Wv8", [P, DB, H * HS], U8, kind="ExternalInput").ap()
    bv_d = nc.dram_tensor("bv", [H, HS], F32, kind="ExternalInput").ap()
    wp8_d = nc.dram_tensor("Wp8", [P, EBS, D], mybir.dt.uint8, kind="ExternalInput").ap()
    bp_d = nc.dram_tensor("bproj", [D], F32, kind="ExternalInput").ap()
    w1c_d = nc.dram_tensor("W1c", [FF // 256, P, DB, 2, 256], U8,
                           kind="ExternalInput").ap()
    b1_d = nc.dram_tensor("b1f", [FF], F32, kind="ExternalInput").ap()
    w2c_d = nc.dram_tensor("W2c", [FO // 2, P, 2, 2, D], U8,
                           kind="ExternalInput").ap()
    b2_d = nc.dram_tensor("b2", [D], F32, kind="ExternalInput").ap()
    out_d = nc.dram_tensor("out", [N, D], F32, kind="ExternalOutput").ap()
    x2pb_d = nc.dram_tensor("x2pb_scratch", [P, NT, D], F32).ap()

    with tile.TileContext(nc) as tc:
        with tc.tile_pool(name="cn", bufs=1) as cp, \
             tc.tile_pool(name="big", bufs=1) as bp:
            # ---- constants / bias vectors (persistent, tiny) ----
            ident = cp.tile([P, P], F32)
            make_identity(nc, ident[:])
            ones_f = cp.tile([P, 1], F32)
            nc.vector.memset(ones_f[:], 1.0)
            ones64 = cp.tile([1, HS], R)
            nc.vector.tensor_copy(ones64[:],
                                  ones_f[0:1, :].to_broadcast([1, HS]))
            onesP = cp.tile([1, P], R)
            nc.vector.tensor_copy(onesP[:],
                                  ones_f[0:1, :].to_broadcast([1, P]))
            epsv = cp.tile([P, 1], F32)
            nc.vector.memset(epsv[:], LN_EPS)
            identR = cp.tile([P, P], R)
            nc.vector.tensor_copy(identR[:], ident[:])
            identR64 = cp.tile([P, P], R)
            nc.vector.tensor_scalar_mul(identR64[:], ident[:], 64.0)
            onesP64 = cp.tile([1, P], R)
            nc.vector.tensor_scalar_mul(onesP64[:], onesP[:], 64.0)
            identR32 = cp.tile([P, P], R)
            nc.vector.tensor_scalar_mul(identR32[:], ident[:], 32.0)
            onesP32 = cp.tile([1, P], R)
            nc.vector.tensor_scalar_mul(onesP32[:], onesP[:], 32.0)
            b2row = cp.tile([1, D], R)
            nc.sync.dma_start(b2row[:], b2_d[None, :].bitcast(R))

            # x load first so the big DMA isn't stuck behind the
            # scattered little bias loads
            xsb = bp.tile([P, NT, D], R, tag="at", name="xsb")
            xr3 = x_d.rearrange("(t p) d -> p t d", p=P)
            for tb in range(NT):
                nc.sync.dma_start(xsb[:, tb, :], xr3[:, tb, :].bitcast(R))

            # biases arrive pre-folded (LN affine) and pre-scaled (x32)
            bqv = cp.tile([P, EBS], F32)
            nc.sync.dma_start(bqv[:], bq_d.rearrange("h s -> (h s)")
                              .rearrange("(b p) -> p b", p=P))
            bkv = cp.tile([P, EBS], F32)
            nc.sync.dma_start(bkv[:], bk_d.rearrange("h s -> (h s)")
                              .rearrange("(b p) -> p b", p=P))
            bvrow = cp.tile([1, H * HS], R)
            nc.sync.dma_start(bvrow[:], bv_d.rearrange("h s -> (h s)")
                              [None, :].bitcast(R))
            b1v = cp.tile([P, FF // P], F32)
            nc.sync.dma_start(b1v[:], b1_d.rearrange("(b p) -> p b", p=P))
            bneg2 = cp.tile([P, 1], F32)
            nc.vector.memset(bneg2[:], -2.0)
            wq8t = bp.tile([P, DB, H * HS], F8, tag="wq8", name="wq8t")
            nc.sync.dma_start(wq8t[:], wq8_d.bitcast(F8))
            wk8t = bp.tile([P, DB, H * HS], F8, tag="wk8", name="wk8t")
            nc.sync.dma_start(wk8t[:], wk8_d.bitcast(F8))
            wv8t = bp.tile([P, DB, H * HS], F8, tag="wv8", name="wv8t")
            nc.sync.dma_start(wv8t[:], wv8_d.bitcast(F8))

            # LN stats scratch (reused for LN2 by tag)
            st_sum = cp.tile([P, NT], F32)
            st_sq = cp.tile([P, NT], F32)
            st_var = cp.tile([P, NT], F32)
            st_rs = cp.tile([P, NT], F32)
            st_nm = cp.tile([P, NT], F32)
            st_vh = cp.tile([P, NT], F32)
            st_t = cp.tile([P, NT], F32)
            st_ih = cp.tile([P, NT], mybir.dt.int32)

            def layernorm_transpose(src, dst, pfx, pspool, trbufs,
                                    delta=False):
                """src: [P, NT, D] token layout (f32) -> dst [P, DB, N] fp8
                feature layout (normalized, no affine -- folded into the
                consumer weights on the host). With delta=True, dst is
                [P, DB, 2, N]: slot 0 = fp8 value, slot 1 = fp8 residual.
                Stats run as one batched 8-wide chain over all token tiles.
                """
                def stats_group(g0, g1):
                    for tb in range(g0, g1):
                        nc.vector.reduce_sum(st_sum[:, tb:tb + 1],
                                             src[:, tb, :],
                                             axis=mybir.AxisListType.X)
                        sq = bp.tile([P, D], F32, tag="qb", bufs=2,
                                     name=f"sq{pfx}")
                        nc.scalar.activation(sq[:], src[:, tb, :], AF.Square,
                                             accum_out=st_sq[:, tb:tb + 1])
                    sm = st_sum[:, g0:g1]
                    var = st_var[:, g0:g1]
                    rs = st_rs[:, g0:g1]
                    nm = st_nm[:, g0:g1]
                    ih = st_ih[:, g0:g1]
                    vh = st_vh[:, g0:g1]
                    tt = st_t[:, g0:g1]
                    i32 = mybir.dt.int32
                    # var = sq/D - (sum/D)^2 + eps   (depth-3 chain)
                    nc.vector.tensor_tensor(tt, sm, sm, OP.mult)
                    nc.vector.tensor_scalar(var, tt, -1.0 / (D * D), LN_EPS,
                                            OP.mult, OP.add)
                    nc.vector.tensor_scalar_mul(tt, st_sq[:, g0:g1], 1.0 / D)
                    nc.vector.tensor_tensor(var, tt, var, OP.add)
                    # rstd = rsqrt(var), DVE-only (bit hack + 2 Newton steps)
                    # so the ACT engine never needs the sqrt table set
                    nc.vector.tensor_scalar(ih, var.bitcast(i32), 1, None,
                                            OP.arith_shift_right)
                    nc.vector.tensor_scalar(rs.bitcast(i32), ih, -1,
                                            0x5F3759DF, OP.mult, OP.add)
                    nc.vector.tensor_scalar_mul(vh, var, -0.5)
                    for _ in range(2):
                        nc.vector.tensor_tensor(tt, rs, rs, OP.mult)
                        nc.vector.tensor_tensor(tt, tt, vh, OP.mult)
                        nc.vector.tensor_scalar(tt, tt, 1.0, 1.5,
                                                OP.mult, OP.add)
                        nc.vector.tensor_tensor(rs, rs, tt, OP.mult)
                    # nm = -(sum/D)*rstd
                    nc.vector.tensor_tensor(nm, sm, rs, OP.mult)
                    nc.vector.tensor_scalar_mul(nm, nm, -1.0 / D)

                for (a, b) in ((0, 1), (1, 2), (2, 4), (4, 8)):
                    stats_group(a, b)
                for tb in range(NT):
                    tnorm = bp.tile([P, D], F32, tag="kb", bufs=2,
                                    name=f"tn{pfx}")
                    nc.gpsimd.tensor_scalar(tnorm[:], src[:, tb, :],
                                             st_rs[:, tb:tb + 1],
                                             st_nm[:, tb:tb + 1],
                                             OP.mult, OP.add)
                    for dh in range(2):
                        pt4 = pspool.tile([P, 4, 512], F32, tag="tr",
                                          bufs=trbufs,
                                          name=f"ptr{pfx}_{tb}_{dh}")
                        for j in range(4):
                            nc.tensor.transpose(pt4[:, j, 0:P],
                                                tnorm[:, ts(dh * 4 + j, P)],
                                                ident[:])
                        d0 = dh * 4
                        if delta:
                            nc.scalar.activation(
                                dst[:, d0:d0 + 4, 0, ts(tb, P)],
                                pt4[:, :, 0:P], AF.Copy)
                            nc.vector.tensor_tensor(
                                dst[:, d0:d0 + 4, 1, ts(tb, P)],
                                pt4[:, :, 0:P],
                                dst[:, d0:d0 + 4, 0, ts(tb, P)],
                                OP.subtract)
                        else:
                            nc.vector.tensor_copy(
                                dst[:, d0:d0 + 4, ts(tb, P)],
                                pt4[:, :, 0:P])

            # ================= Phase A: LN1 + transpose ====================
            HT = bp.tile([P, DB, N], F8, tag="h8", name="HT")
            psAB_cm = tc.tile_pool(name="psAB", bufs=1, space="PSUM")
            psAB = psAB_cm.__enter__()
            layernorm_transpose(xsb, HT, "a", psAB, 1)

            # ================= Phase B0: V projection ======================
            Vaug = bp.tile([P, NT, H, HS + 1], F8, tag="v8", name="Vaug")
            nc.vector.tensor_copy(
                Vaug[:, :, :, HS:HS + 1],
                ones_f[:, None, :].to_broadcast([P, NT, H, 1]))
            if True:
                for eh in range(2):
                    for tb in range(NT):
                        pv = psAB.tile([P, 512], F32, tag="qkv", bufs=4,
                                       name=f"pv{eh}_{tb}")
                        for kp in range(DB // 2):
                            nc.tensor.matmul(
                                pv[:], HT[:, 2 * kp:2 * kp + 2, ts(tb, P)],
                                wv8t[:, 2 * kp:2 * kp + 2,
                                     ds(eh * 512, 512)],
                                start=(kp == 0), stop=False, perf_mode=DRM)
                        # += 32*bv' via ones row (psum carries 32x values)
                        nc.tensor.matmul(pv[:], onesP[:],
                                         bvrow[:, ds(eh * 512, 512)],
                                         start=False, stop=True)
                        nc.scalar.activation(
                            Vaug[:, tb, eh * 8:(eh + 1) * 8, 0:HS],
                            pv[:].rearrange("p (h s) -> p h s", s=HS), AF.Copy,
                            scale=1.0 / 32.0)

            # ============ Phase BC: Q/K per e-block fused with attention ===
            psAB_cm.__exit__(None, None, None)
            psBC_cm = tc.tile_pool(name="psBC", bufs=1, space="PSUM")
            psBC = psBC_cm.__enter__()
            attnT8 = bp.tile([P, EBS, N], F8, tag="a8", name="attnT8")
            # prefetch Wproj (fp8) during attention
            wp8 = bp.tile([P, EBS, D], F8, tag="sh", bufs=1, name="wp8")
            nc.sync.dma_start(wp8[:], wp8_d.bitcast(F8))

            if True:
                for eb in range(EBS):
                    Qb = bp.tile([P, N], R, tag="qb", bufs=2, name=f"Qb{eb}")
                    Kb = bp.tile([P, N], R, tag="kb", bufs=2, name=f"Kb{eb}")
                    ecs = ds(eb * P, P)
                    for nh in range(NH):
                        pq = psBC.tile([P, 512], F32, tag="qk", bufs=2,
                                       name=f"pq{eb}_{nh}")
                        for kp in range(DB // 2):
                            nc.tensor.matmul(pq[:],
                                             wq8t[:, 2 * kp:2 * kp + 2, ecs],
                                             HT[:, 2 * kp:2 * kp + 2,
                                                ds(nh * 512, 512)],
                                             start=(kp == 0), stop=(kp == 3),
                                             perf_mode=DRM)
                        nc.vector.tensor_scalar_add(Qb[:, ds(nh * 512, 512)],
                                                    pq[:], bqv[:, eb:eb + 1])
                        pk = psBC.tile([P, 512], F32, tag="qk", bufs=2,
                                       name=f"pk{eb}_{nh}")
                        for kp in range(DB // 2):
                            nc.tensor.matmul(pk[:],
                                             wk8t[:, 2 * kp:2 * kp + 2, ecs],
                                             HT[:, 2 * kp:2 * kp + 2,
                                                ds(nh * 512, 512)],
                                             start=(kp == 0), stop=(kp == 3),
                                             perf_mode=DRM)
                        nc.vector.tensor_scalar_add(Kb[:, ds(nh * 512, 512)],
                                                    pk[:], bkv[:, eb:eb + 1])

                    # attention for heads 2eb (partitions 0:64) and
                    # 2eb+1 (partitions 64:128), per n-half of 512
                    for nh in range(NH):
                        pts = [bp.tile([P, NT, 512], F8, tag="p8", bufs=2,
                                       name=f"PT{eb}_{nh}_{i}")
                               for i in range(2)]
                        # scoresT[m, n] = sum_s K[m,s] Q[n,s]; exp via ACT
                        # (Qb/Kb carry 32x values -> scale 0.125/1024); the
                        # -2 bias keeps exp within fp8 range, cancels in the
                        # softmax ratio
                        for mq in range(NT // 2):
                            for i in range(2):
                                base = i * HS
                                pss = psBC.tile([P, 2, 512], F32, tag="sc",
                                                bufs=2,
                                                name=f"ps{eb}{nh}{mq}{i}")
                                for mj in range(2):
                                    nc.tensor.matmul(
                                        pss[:, mj, :],
                                        Kb[base:base + HS,
                                           ts(mq * 2 + mj, P)],
                                        Qb[base:base + HS,
                                           ds(nh * 512, 512)],
                                        start=True, stop=True)
                                nc.scalar.activation(
                                    pts[i][:, 2 * mq:2 * mq + 2, :], pss[:],
                                    AF.Exp, scale=0.125 / 1024.0,
                                    bias=bneg2[:])
                        pas = [psBC.tile([HS + 1, 512], F32, tag="at65",
                                         bufs=2, name=f"pa{eb}_{nh}_{i}")
                               for i in range(2)]
                        for mp in range(NT // 2):
                            for i in range(2):
                                nc.tensor.matmul(pas[i][:],
                                                 Vaug[:, 2 * mp:2 * mp + 2,
                                                      2 * eb + i, :],
                                                 pts[i][:, 2 * mp:2 * mp + 2,
                                                        :],
                                                 start=(mp == 0),
                                                 stop=(mp == NT // 2 - 1),
                                                 perf_mode=DRM)
                        for i in range(2):
                            base = i * HS
                            rec = bp.tile([1, 512], F32, tag="rb", bufs=4,
                                          name=f"rc{eb}_{nh}_{i}")
                            nc.vector.reciprocal(rec[:],
                                                 pas[i][HS:HS + 1, :])
                            rbs = bp.tile([HS, 512], F32, tag="rb", bufs=4,
                                          name=f"rb{eb}_{nh}_{i}")
                            nc.gpsimd.partition_broadcast(rbs[:], rec[:])
                            dstA = attnT8[base:base + HS, eb,
                                          ds(nh * 512, 512)]
                            nc.vector.tensor_tensor(dstA, pas[i][0:HS, :],
                                                    rbs[:], OP.mult)

            # w1(ft0) prefetch into "se" (frees at end of attention);
            # high priority so the DMA issues as soon as the slot frees
            w1pre = bp.tile([P, DB, 2, 256], F8, tag="w1s", bufs=3,
                            name="w1pre")
            with tc.high_priority():
                nc.gpsimd.dma_start(w1pre[:], w1c_d[0].bitcast(F8))

            # ================= Phase D: proj + residual ====================
            psBC_cm.__exit__(None, None, None)
            psDE_cm = tc.tile_pool(name="psDE", bufs=1, space="PSUM")
            psDE = psDE_cm.__enter__()
            x2 = bp.tile([P, NT, D], R, tag="ht", name="x2")
            if True:
                bprow = bp.tile([1, D], R, tag="kb", bufs=2, name="bprow")
                nc.sync.dma_start(bprow[:], bp_d[None, :].bitcast(R))
                for tb in range(NT):
                    for dt in range(2):
                        pp = psDE.tile([P, 512], F32, tag="pj", bufs=4,
                                       name=f"pp{tb}_{dt}")
                        for gp in range(4):
                            nc.tensor.matmul(
                                pp[:],
                                attnT8[:, 2 * gp:2 * gp + 2, ts(tb, P)],
                                wp8[:, 2 * gp:2 * gp + 2, ds(dt * 512, 512)],
                                start=(gp == 0), stop=False, perf_mode=DRM)
                        # bias + residual folded into the PE accumulation
                        # (psum carries 32x the true values)
                        nc.tensor.matmul(pp[:], onesP32[:],
                                         bprow[:, ds(dt * 512, 512)],
                                         start=False, stop=False)
                        nc.tensor.matmul(pp[:], identR32[:],
                                         xsb[:, tb, ds(dt * 512, 512)],
                                         start=False, stop=True)
                        nc.scalar.activation(x2[:, tb, ds(dt * 512, 512)],
                                             pp[:], AF.Copy,
                                             scale=1.0 / 32.0)

            # ============ Phase E: LN2 (affine folded into W1/b1 on host),
            # transpose into fp8 pair (h28, dh28); stash x2+b2 ==============
            H2x = bp.tile([P, DB, 2, N], F8, tag="se", bufs=1, name="H2x")
            if True:
                layernorm_transpose(x2, H2x, "e", psDE, 1, delta=True)

            # ================= Phase F: FFN (fp8 DR, 3-chain corrected) ====
            psDE_cm.__exit__(None, None, None)
            psF_cm = tc.tile_pool(name="psF", bufs=1, space="PSUM")
            psF = psF_cm.__enter__()
            if True:
                def ffn1_group(nt, ft, fc, pool, ptag, pbufs, w1t):
                    p1 = pool.tile([P, 512], F32, tag=ptag, bufs=pbufs,
                                   name=f"p1_{nt}_{ft}_{fc}")
                    fcs = ds(fc * P, P)
                    n12 = 0
                    for ch in range(3):
                        wsl, hsl = ((0, 0), (1, 0), (0, 1))[ch]
                        for kp in range(DB // 2):
                            nc.tensor.matmul(
                                p1[:],
                                w1t[:, 2 * kp:2 * kp + 2, wsl, fcs],
                                H2x[:, 2 * kp:2 * kp + 2, hsl,
                                    ds(nt * 512, 512)],
                                start=(n12 == 0), stop=(n12 == 11),
                                perf_mode=DRM)
                            n12 += 1
                    bf = ft * 2 + fc
                    if use_lrelu:
                        mz = bp.tile([P, 512], F32, tag="rb", bufs=4,
                                     name=f"mz{nt}_{bf}")
                        nc.scalar.activation(mz[:], p1[:], AF.Prelu,
                                             bias=b1v[:, bf:bf + 1],
                                             scale=1.0 / SW1,
                                             alpha=0.01)
                    else:
                        z = bp.tile([P, 512], F32, tag="qb", bufs=2,
                                    name=f"z{nt}_{bf}")
                        nc.scalar.activation(z[:], p1[:], AF.Identity,
                                             bias=b1v[:, bf:bf + 1],
                                             scale=1.0 / SW1)
                        zs = bp.tile([P, 512], F32, tag="rb", bufs=4,
                                     name=f"zs{nt}_{bf}")
                        nc.vector.tensor_scalar_mul(zs[:], z[:], 0.01)
                        mz = bp.tile([P, 512], F32, tag="rb", bufs=4,
                                     name=f"mz{nt}_{bf}")
                        nc.vector.tensor_tensor(mz[:], z[:], zs[:], OP.max)
                    # quantize on Pool; residual sub on DVE (Pool was the
                    # FFN1 pacing engine)
                    nc.vector.tensor_copy(zx[:, bf, 0, :], mz[:])
                    nc.vector.tensor_tensor(zx[:, bf, 1, :], mz[:],
                                            zx[:, bf, 0, :], OP.subtract)

                for nt in range(NH):
                    zx = bp.tile([P, FO, 2, 512], F8, tag="at",
                                 name=f"zx{nt}")
                    for ft in range(FF // 256):
                        if ft == 0 and nt == 0:
                            w1t = w1pre
                        else:
                            w1t = bp.tile([P, DB, 2, 256], F8, tag="w1s",
                                          bufs=3, name=f"w1_{nt}_{ft}")
                            nc.gpsimd.dma_start(w1t[:],
                                                w1c_d[ft].bitcast(F8))
                        for fc in range(2):
                            ffn1_group(nt, ft, fc, psF, "fp", 8, w1t)
                    pf2 = [psF.tile([P, 512], F32, tag="fp", bufs=8,
                                    name=f"p2_{nt}_{j}") for j in range(8)]
                    NG = FO // 2   # 16 fo-pairs

                    def w2_pair(nt, g):
                        w2t = bp.tile([P, 2, 2, D], F8, tag="w2s", bufs=3,
                                      name=f"w2_{nt}_{g}")
                        nc.gpsimd.dma_start(w2t[:], w2c_d[g].bitcast(F8))
                        return w2t

                    def ffn2_chains(g, w2t, tb, dt, start):
                        for ch in range(3):
                            if ch == 1 and g >= NG // 2:
                                # dW2 correction on half the f-range is
                                # enough for the error budget
                                continue
                            wsl, zsl = ((0, 0), (1, 0), (0, 1))[ch]
                            nc.tensor.matmul(
                                pf2[tb * 2 + dt][:],
                                zx[:, 2 * g:2 * g + 2, zsl, ts(tb, P)],
                                w2t[:, :, wsl, ds(dt * 512, 512)],
                                start=(start and ch == 0), stop=False,
                                perf_mode=DRM)

                    for g in range(NG - 1):
                        w2t = w2_pair(nt, g)
                        for tb in range(4):
                            for dt in range(2):
                                ffn2_chains(g, w2t, tb, dt, g == 0)
                    # last fo-pair: close each psum group in turn; fold the
                    # residual (64*(x2+b2)) via identity matmul, evac 1/64
                    gl = NG - 1
                    w2t = w2_pair(nt, gl)
                    for tb in range(4):
                        for dt in range(2):
                            ffn2_chains(gl, w2t, tb, dt, False)
                            nc.tensor.matmul(
                                pf2[tb * 2 + dt][:], onesP64[:],
                                b2row[:, ds(dt * 512, 512)],
                                start=False, stop=False)
                            nc.tensor.matmul(
                                pf2[tb * 2 + dt][:], identR64[:],
                                x2[:, nt * 4 + tb, ds(dt * 512, 512)],
                                start=False, stop=True)
                            rows = ds(nt * 512 + tb * P, P)
                            og = bp.tile([P, 512], F32, tag="rb", bufs=4,
                                         name=f"og{nt}_{tb}_{dt}")
                            nc.scalar.activation(og[:], pf2[tb * 2 + dt][:],
                                                 AF.Copy, scale=1.0 / 64.0)
                            nc.sync.dma_start(out_d[rows, ds(dt * 512, 512)],
                                              og[:])
            psF_cm.__exit__(None, None, None)
    nc.compile()
    return nc


def get_nc():
    global _CACHED_NC
    if _CACHED_NC is None:
        _CACHED_NC = build_nc()
    return _CACHED_NC


def _q8pair(w, s):
    """fp8(s*w) and same-scale fp8 residual, as uint8 bit views."""
    hi = (s * w).astype(E4M3)
    lo = (s * w - hi.astype(np.float32)).astype(E4M3)
    return (np.ascontiguousarray(hi).view(np.uint8),
            np.ascontiguousarray(lo).view(np.uint8))


def prep_weights(inputs):
    f32 = lambda k: np.asarray(inputs[k], dtype=np.float64)
    g2, c2 = f32("ln2_g"), f32("ln2_b")
    W1, b1, W2 = f32("W1"), f32("b1"), f32("W2")
    # fold LN2 affine into W1/b1
    W1f = W1 * g2[:, None]
    b1f = (b1 + c2 @ W1).astype(np.float32)
    w1l = W1f.reshape(DB, P, FF).transpose(1, 0, 2).astype(np.float32)
    w2l = W2.reshape(FO, P, D).transpose(1, 0, 2).astype(np.float32)
    W18, dW18 = _q8pair(w1l, SW1)   # [P, DB, FF] uint8 views
    W28, dW28 = _q8pair(w2l, SW2)   # [P, FO, D]
    # pack (W, dW) pairs into per-tile contiguous blobs
    w1s = np.stack([W18, dW18], axis=2)            # [P, DB, 2, FF]
    W1c = np.ascontiguousarray(
        w1s.reshape(P, DB, 2, FF // 256, 256)
        .transpose(3, 0, 1, 2, 4))                 # [16, P, DB, 2, 256]
    w2s_ = np.stack([W28, dW28], axis=2)           # [P, FO, 2, D]
    W2c = np.ascontiguousarray(
        w2s_.reshape(P, FO // 2, 2, 2, D)
        .transpose(1, 0, 2, 3, 4))                 # [16, P, 2, 2, D]
    Wp = f32("Wproj")
    Wp8 = np.ascontiguousarray(
        (SWP * Wp.reshape(EBS, P, D).transpose(1, 0, 2))
        .astype(E4M3)).view(np.uint8)
    g1, c1 = f32("ln1_g"), f32("ln1_b")

    def qkvfold(wname, bname):
        W, b = f32(wname), f32(bname)          # [H, D, HS], [H, HS]
        Wf = W * g1[None, :, None]
        bf = b + np.einsum('d,hds->hs', c1, W)
        wl = (Wf.transpose(1, 0, 2).reshape(DB, P, H * HS)
              .transpose(1, 0, 2).astype(np.float32))
        w8 = np.ascontiguousarray((SWP * wl).astype(E4M3)).view(np.uint8)
        return w8, np.ascontiguousarray((SWP * bf).astype(np.float32))

    Wq8, bq32 = qkvfold("Wq", "bq")
    Wk8, bk32 = qkvfold("Wk", "bk")
    Wv8, bv32 = qkvfold("Wv", "bv")
    w = {k: np.ascontiguousarray(np.asarray(inputs[k], dtype=np.float32))
         for k in ("bproj", "b2")}
    w.update(W1c=W1c, b1f=b1f, W2c=W2c, Wp8=Wp8,
             Wq8=Wq8, bq=bq32, Wk8=Wk8, bk=bk32, Wv8=Wv8, bv=bv32)
    return w


def kernel(**inputs):
    nc = get_nc()
    x = np.ascontiguousarray(np.asarray(inputs["x"], dtype=np.float32))
    B = x.shape[0]
    weights = prep_weights(inputs)
    in_maps = [dict(weights, x=x[b]) for b in range(B)]
    res = run_bass_kernel_spmd(nc, in_maps, list(range(B)))
    return np.stack([res.results[b]["out"] for b in range(B)], axis=0)



# revision 48
# speedup vs baseline: 1.3535x; 1.0440x over previous
"""Trainium2 Bass kernel for a pre-LN transformer block (MHA + FFN).

Data-parallel over batch: 8 NeuronCores, one batch element each.

Speed comes from fp8(e4m3) DoubleRow matmuls (0.5 PE cycles/row over
k-tile pairs = 4x the f32r MAC rate), with precision recovered where it
matters:
  - QKV / attnV / proj run plain fp8 DR (their noise contribution to the
    output is tiny, measured ~5e-3 combined);
  - FFN1/FFN2 run "3-chain corrected" DR:
        y @ W ~= y8@W8 + y8@dW8 + dy8@W8
    where dW8/dy8 are same-scale fp8 residuals (subnormal range), giving
    ~0.1% noise at 0.75x the f32r cycle count. FFN2's dW chain covers
    only half the f-range (error budget allows it).
  - attention scores stay f32r; softmax exp runs on ACT with a -2 bias
    so e^s fits fp8, the shift cancels in the softmax ratio.

Host-side prep (free): LN affines folded into the consumer weights and
biases, weights quantized + packed per-tile-contiguous (W|dW
interleaved), biases pre-scaled by the fp8 weight scales.

On-device structure: LN1 -> fp8 transpose; V/Q/K DR projections with
bias folded into the PE accumulation via ones-row matmuls; per-head-pair
attention (f32r scores, batched exp->fp8, DR attnV with an appended
ones row producing the softmax denominators); DR proj with residual +
bias accumulated on the PE (identity/ones matmuls); LN2 -> fp8 value +
residual pair; FFN with prelu on ACT, z-quantization on Pool/DVE, and
the residual+b2 again folded into the final PE accumulation.
"""
import sys

for _p in ("/opt/trn_rl_repo", "/root/.axon_site/_ro/trn_rl_repo"):
    if _p not in sys.path:
        sys.path.insert(0, _p)

import numpy as np
import ml_dtypes
import concourse.bass as bass
import concourse.tile as tile
from concourse import bacc, mybir
from concourse.bass import ds, ts
from concourse.bass_utils import run_bass_kernel_spmd
from concourse.masks import make_identity

E4M3 = (ml_dtypes.float8_e4m3fn if hasattr(ml_dtypes, "float8_e4m3fn")
        else ml_dtypes.float8_e4m3)
SW1 = 32.0      # host scale for W1 fp8
SWP = 32.0      # host scale for Wproj fp8
SW2 = 64.0      # host scale for W2 fp8

P = 128
N = 1024          # tokens per core (seq len)
D = 1024          # d_emb
H = 16            # heads
HS = 64           # head size
FF = 4096         # ffn hidden
NT = N // P       # 8 token tiles
DB = D // P       # 8 d blocks
EBS = D // P      # 8 e blocks (qkv out features)
NH = 2            # n halves of 512
LN_EPS = 1e-5

F32 = mybir.dt.float32
R = mybir.dt.float32r
F8 = mybir.dt.float8e4
U8 = mybir.dt.uint8
AF = mybir.ActivationFunctionType
OP = mybir.AluOpType
DRM = mybir.MatmulPerfMode.DoubleRow
FO = FF // P      # 32 f-blocks for FFN2 contraction

_CACHED_NC = None


def build_nc(use_lrelu=True):
    nc = bacc.Bacc("TRN2", target_bir_lowering=False, debug=False, num_devices=8)

    x_d = nc.dram_tensor("x", [N, D], F32, kind="ExternalInput").ap()
    wq8_d = nc.dram_tensor("Wq8", [P, DB, H * HS], U8, kind="ExternalInput").ap()
    bq_d = nc.dram_tensor("bq", [H, HS], F32, kind="ExternalInput").ap()
    wk8_d = nc.dram_tensor("Wk8", [P, DB, H * HS], U8, kind="ExternalInput").ap()
    bk_d = nc.dram_tensor("bk", [H, HS], F32, kind="ExternalInput").ap()
    wv8_d = nc.dram_tensor("Wv8", [P, DB, H * HS], U8, kind="ExternalInput").ap()
    bv_d = nc.dram_tensor("bv", [H, HS], F32, kind="ExternalInput").ap()
    wp8_d = nc.dram_tensor("Wp8", [P, EBS, D], mybir.dt.uint8, kind="ExternalInput").ap()
    bp_d = nc.dram_tensor("bproj", [D], F32, kind="ExternalInput").ap()
    w1c_d = nc.dram_tensor("W1c", [FF // 256, P, DB, 2, 256], U8,
                           kind="ExternalInput").ap()
    b1_d = nc.dram_tensor("b1f", [FF], F32, kind="ExternalInput").ap()
    w2c_d = nc.dram_tensor("W2c", [FO // 2, P, 2, 2, D], U8,
                           kind="ExternalInput").ap()
    b2_d = nc.dram_tensor("b2", [D], F32, kind="ExternalInput").ap()
    out_d = nc.dram_tensor("out", [N, D], F32, kind="ExternalOutput").ap()
    x2pb_d = nc.dram_tensor("x2pb_scratch", [P, NT, D], F32).ap()

    with tile.TileContext(nc) as tc:
        with tc.tile_pool(name="cn", bufs=1) as cp, \
             tc.tile_pool(name="big", bufs=1) as bp:
            # ---- constants / bias vectors (persistent, tiny) ----
            ident = cp.tile([P, P], F32)
            make_identity(nc, ident[:])
            ones_f = cp.tile([P, 1], F32)
            nc.vector.memset(ones_f[:], 1.0)
            ones64 = cp.tile([1, HS], R)
            nc.vector.tensor_copy(ones64[:],
                                  ones_f[0:1, :].to_broadcast([1, HS]))
            onesP = cp.tile([1, P], R)
            nc.vector.tensor_copy(onesP[:],
                                  ones_f[0:1, :].to_broadcast([1, P]))
            epsv = cp.tile([P, 1], F32)
            nc.vector.memset(epsv[:], LN_EPS)
            identR = cp.tile([P, P], R)
            nc.vector.tensor_copy(identR[:], ident[:])
            identR64 = cp.tile([P, P], R)
            nc.vector.tensor_scalar_mul(identR64[:], ident[:], 64.0)
            onesP64 = cp.tile([1, P], R)
            nc.vector.tensor_scalar_mul(onesP64[:], onesP[:], 64.0)
            identR32 = cp.tile([P, P], R)
            nc.vector.tensor_scalar_mul(identR32[:], ident[:], 32.0)
            onesP32 = cp.tile([1, P], R)
            nc.vector.tensor_scalar_mul(onesP32[:], onesP[:], 32.0)
            b2row = cp.tile([1, D], R)
            nc.sync.dma_start(b2row[:], b2_d[None, :].bitcast(R))

            # x load first so the big DMA isn't stuck behind the
            # scattered little bias loads
            xsb = bp.tile([P, NT, D], R, tag="at", name="xsb")
            xr3 = x_d.rearrange("(t p) d -> p t d", p=P)
            for tb in range(NT):
                nc.sync.dma_start(xsb[:, tb, :], xr3[:, tb, :].bitcast(R))

            # biases arrive pre-folded (LN affine) and pre-scaled (x32)
            bqv = cp.tile([P, EBS], F32)
            nc.sync.dma_start(bqv[:], bq_d.rearrange("h s -> (h s)")
                              .rearrange("(b p) -> p b", p=P))
            bkv = cp.tile([P, EBS], F32)
            nc.sync.dma_start(bkv[:], bk_d.rearrange("h s -> (h s)")
                              .rearrange("(b p) -> p b", p=P))
            bvrow = cp.tile([1, H * HS], R)
            nc.sync.dma_start(bvrow[:], bv_d.rearrange("h s -> (h s)")
                              [None, :].bitcast(R))
            b1v = cp.tile([P, FF // P], F32)
            nc.sync.dma_start(b1v[:], b1_d.rearrange("(b p) -> p b", p=P))
            bneg2 = cp.tile([P, 1], F32)
            nc.vector.memset(bneg2[:], -2.0)
            wv8t = bp.tile([P, DB, H * HS], F8, tag="wv8", name="wv8t")
            nc.sync.dma_start(wv8t[:], wv8_d.bitcast(F8))
            wq8t = bp.tile([P, DB, H * HS], F8, tag="wq8", name="wq8t")
            nc.sync.dma_start(wq8t[:], wq8_d.bitcast(F8))
            wk8t = bp.tile([P, DB, H * HS], F8, tag="wk8", name="wk8t")
            nc.sync.dma_start(wk8t[:], wk8_d.bitcast(F8))

            # LN stats scratch (reused for LN2 by tag)
            st_sum = cp.tile([P, NT], F32)
            st_sq = cp.tile([P, NT], F32)
            st_var = cp.tile([P, NT], F32)
            st_rs = cp.tile([P, NT], F32)
            st_nm = cp.tile([P, NT], F32)
            st_vh = cp.tile([P, NT], F32)
            st_t = cp.tile([P, NT], F32)
            st_ih = cp.tile([P, NT], mybir.dt.int32)

            def layernorm_transpose(src, dst, pfx, pspool, trbufs,
                                    delta=False):
                """src: [P, NT, D] token layout (f32) -> dst [P, DB, N] fp8
                feature layout (normalized, no affine -- folded into the
                consumer weights on the host). With delta=True, dst is
                [P, DB, 2, N]: slot 0 = fp8 value, slot 1 = fp8 residual.
                Stats run as one batched 8-wide chain over all token tiles.
                """
                def stats_group(g0, g1):
                    for tb in range(g0, g1):
                        nc.vector.reduce_sum(st_sum[:, tb:tb + 1],
                                             src[:, tb, :],
                                             axis=mybir.AxisListType.X)
                        sq = bp.tile([P, D], F32, tag="qb", bufs=2,
                                     name=f"sq{pfx}")
                        nc.scalar.activation(sq[:], src[:, tb, :], AF.Square,
                                             accum_out=st_sq[:, tb:tb + 1])
                    sm = st_sum[:, g0:g1]
                    var = st_var[:, g0:g1]
                    rs = st_rs[:, g0:g1]
                    nm = st_nm[:, g0:g1]
                    ih = st_ih[:, g0:g1]
                    vh = st_vh[:, g0:g1]
                    tt = st_t[:, g0:g1]
                    i32 = mybir.dt.int32
                    # var = sq/D - (sum/D)^2 + eps   (depth-3 chain)
                    nc.vector.tensor_tensor(tt, sm, sm, OP.mult)
                    nc.vector.tensor_scalar(var, tt, -1.0 / (D * D), LN_EPS,
                                            OP.mult, OP.add)
                    nc.vector.tensor_scalar_mul(tt, st_sq[:, g0:g1], 1.0 / D)
                    nc.vector.tensor_tensor(var, tt, var, OP.add)
                    # rstd = rsqrt(var), DVE-only (bit hack + 2 Newton steps)
                    # so the ACT engine never needs the sqrt table set
                    nc.vector.tensor_scalar(ih, var.bitcast(i32), 1, None,
                                            OP.arith_shift_right)
                    nc.vector.tensor_scalar(rs.bitcast(i32), ih, -1,
                                            0x5F3759DF, OP.mult, OP.add)
                    nc.vector.tensor_scalar_mul(vh, var, -0.5)
                    for _ in range(2):
                        nc.vector.tensor_tensor(tt, rs, rs, OP.mult)
                        nc.vector.tensor_tensor(tt, tt, vh, OP.mult)
                        nc.vector.tensor_scalar(tt, tt, 1.0, 1.5,
                                                OP.mult, OP.add)
                        nc.vector.tensor_tensor(rs, rs, tt, OP.mult)
                    # nm = -(sum/D)*rstd
                    nc.vector.tensor_tensor(nm, sm, rs, OP.mult)
                    nc.vector.tensor_scalar_mul(nm, nm, -1.0 / D)

                for (a, b) in ((0, 1), (1, 2), (2, 4), (4, 8)):
                    stats_group(a, b)
                for tb in range(NT):
                    tnorm = bp.tile([P, D], F32, tag="kb", bufs=2,
                                    name=f"tn{pfx}")
                    nc.gpsimd.tensor_scalar(tnorm[:], src[:, tb, :],
                                             st_rs[:, tb:tb + 1],
                                             st_nm[:, tb:tb + 1],
                                             OP.mult, OP.add)
                    for dh in range(4):
                        pt4 = pspool.tile([P, 2, 512], F32, tag="tr",
                                          bufs=2,
                                          name=f"ptr{pfx}_{tb}_{dh}")
                        for j in range(2):
                            nc.tensor.transpose(pt4[:, j, 0:P],
                                                tnorm[:, ts(dh * 2 + j, P)],
                                                ident[:])
                        d0 = dh * 2
                        if delta:
                            nc.scalar.activation(
                                dst[:, d0:d0 + 2, 0, ts(tb, P)],
                                pt4[:, :, 0:P], AF.Copy)
                            nc.vector.tensor_tensor(
                                dst[:, d0:d0 + 2, 1, ts(tb, P)],
                                pt4[:, :, 0:P],
                                dst[:, d0:d0 + 2, 0, ts(tb, P)],
                                OP.subtract)
                        else:
                            nc.vector.tensor_copy(
                                dst[:, d0:d0 + 2, ts(tb, P)],
                                pt4[:, :, 0:P])

            # ================= Phase A: LN1 + transpose ====================
            HT = bp.tile([P, DB, N], F8, tag="h8", name="HT")
            psAB_cm = tc.tile_pool(name="psAB", bufs=1, space="PSUM")
            psAB = psAB_cm.__enter__()
            layernorm_transpose(xsb, HT, "a", psAB, 1)

            # ================= Phase B0: V projection ======================
            Vaug = bp.tile([P, NT, H, HS + 1], F8, tag="v8", name="Vaug")
            nc.vector.tensor_copy(
                Vaug[:, :, :, HS:HS + 1],
                ones_f[:, None, :].to_broadcast([P, NT, H, 1]))
            if True:
                for eh in range(2):
                    for tb in range(NT):
                        pv = psAB.tile([P, 512], F32, tag="qkv", bufs=4,
                                       name=f"pv{eh}_{tb}")
                        for kp in range(DB // 2):
                            nc.tensor.matmul(
                                pv[:], HT[:, 2 * kp:2 * kp + 2, ts(tb, P)],
                                wv8t[:, 2 * kp:2 * kp + 2,
                                     ds(eh * 512, 512)],
                                start=(kp == 0), stop=False, perf_mode=DRM)
                        # += 32*bv' via ones row (psum carries 32x values)
                        nc.tensor.matmul(pv[:], onesP[:],
                                         bvrow[:, ds(eh * 512, 512)],
                                         start=False, stop=True)
                        nc.scalar.activation(
                            Vaug[:, tb, eh * 8:(eh + 1) * 8, 0:HS],
                            pv[:].rearrange("p (h s) -> p h s", s=HS), AF.Copy,
                            scale=1.0 / 32.0)

            # ============ Phase BC: Q/K per e-block fused with attention ===
            psAB_cm.__exit__(None, None, None)
            psBC_cm = tc.tile_pool(name="psBC", bufs=1, space="PSUM")
            psBC = psBC_cm.__enter__()
            attnT8 = bp.tile([P, EBS, N], F8, tag="a8", name="attnT8")
            # prefetch Wproj (fp8) during attention
            wp8 = bp.tile([P, EBS, D], F8, tag="sh", bufs=1, name="wp8")
            nc.sync.dma_start(wp8[:], wp8_d.bitcast(F8))

            if True:
                for eb in range(EBS):
                    Qb = bp.tile([P, N], R, tag="qb", bufs=2, name=f"Qb{eb}")
                    Kb = bp.tile([P, N], R, tag="kb", bufs=2, name=f"Kb{eb}")
                    ecs = ds(eb * P, P)
                    for nh in range(NH):
                        pq = psBC.tile([P, 512], F32, tag="qk", bufs=2,
                                       name=f"pq{eb}_{nh}")
                        for kp in range(DB // 2):
                            nc.tensor.matmul(pq[:],
                                             wq8t[:, 2 * kp:2 * kp + 2, ecs],
                                             HT[:, 2 * kp:2 * kp + 2,
                                                ds(nh * 512, 512)],
                                             start=(kp == 0), stop=(kp == 3),
                                             perf_mode=DRM)
                        nc.vector.tensor_scalar_add(Qb[:, ds(nh * 512, 512)],
                                                    pq[:], bqv[:, eb:eb + 1])
                        pk = psBC.tile([P, 512], F32, tag="qk", bufs=2,
                                       name=f"pk{eb}_{nh}")
                        for kp in range(DB // 2):
                            nc.tensor.matmul(pk[:],
                                             wk8t[:, 2 * kp:2 * kp + 2, ecs],
                                             HT[:, 2 * kp:2 * kp + 2,
                                                ds(nh * 512, 512)],
                                             start=(kp == 0), stop=(kp == 3),
                                             perf_mode=DRM)
                        nc.vector.tensor_scalar_add(Kb[:, ds(nh * 512, 512)],
                                                    pk[:], bkv[:, eb:eb + 1])

                    # attention for heads 2eb (partitions 0:64) and
                    # 2eb+1 (partitions 64:128), per n-half of 512
                    for nh in range(NH):
                        pts = [bp.tile([P, NT, 512], F8, tag="p8", bufs=2,
                                       name=f"PT{eb}_{nh}_{i}")
                               for i in range(2)]
                        # scoresT[m, n] = sum_s K[m,s] Q[n,s]; exp via ACT
                        # (Qb/Kb carry 32x values -> scale 0.125/1024); the
                        # -2 bias keeps exp within fp8 range, cancels in the
                        # softmax ratio
                        for mq in range(NT // 2):
                            for i in range(2):
                                base = i * HS
                                pss = psBC.tile([P, 2, 512], F32, tag="sc",
                                                bufs=2,
                                                name=f"ps{eb}{nh}{mq}{i}")
                                for mj in range(2):
                                    nc.tensor.matmul(
                                        pss[:, mj, :],
                                        Kb[base:base + HS,
                                           ts(mq * 2 + mj, P)],
                                        Qb[base:base + HS,
                                           ds(nh * 512, 512)],
                                        start=True, stop=True)
                                nc.scalar.activation(
                                    pts[i][:, 2 * mq:2 * mq + 2, :], pss[:],
                                    AF.Exp, scale=0.125 / 1024.0,
                                    bias=bneg2[:])
                        pas = [psBC.tile([HS + 1, 512], F32, tag="at65",
                                         bufs=2, name=f"pa{eb}_{nh}_{i}")
                               for i in range(2)]
                        for mp in range(NT // 2):
                            for i in range(2):
                                nc.tensor.matmul(pas[i][:],
                                                 Vaug[:, 2 * mp:2 * mp + 2,
                                                      2 * eb + i, :],
                                                 pts[i][:, 2 * mp:2 * mp + 2,
                                                        :],
                                                 start=(mp == 0),
                                                 stop=(mp == NT // 2 - 1),
                                                 perf_mode=DRM)
                        for i in range(2):
                            base = i * HS
                            rec = bp.tile([1, 512], F32, tag="rb", bufs=4,
                                          name=f"rc{eb}_{nh}_{i}")
                            nc.vector.reciprocal(rec[:],
                                                 pas[i][HS:HS + 1, :])
                            rbs = bp.tile([HS, 512], F32, tag="rb", bufs=4,
                                          name=f"rb{eb}_{nh}_{i}")
                            nc.gpsimd.partition_broadcast(rbs[:], rec[:])
                            dstA = attnT8[base:base + HS, eb,
                                          ds(nh * 512, 512)]
                            nc.vector.tensor_tensor(dstA, pas[i][0:HS, :],
                                                    rbs[:], OP.mult)

            # w1(ft0) prefetch into "se" (frees at end of attention);
            # high priority so the DMA issues as soon as the slot frees
            w1pre = bp.tile([P, DB, 2, 256], F8, tag="w1s", bufs=3,
                            name="w1pre")
            with tc.high_priority():
                nc.gpsimd.dma_start(w1pre[:], w1c_d[0].bitcast(F8))

            # ================= Phase D: proj + residual ====================
            psBC_cm.__exit__(None, None, None)
            psDE_cm = tc.tile_pool(name="psDE", bufs=1, space="PSUM")
            psDE = psDE_cm.__enter__()
            x2 = bp.tile([P, NT, D], R, tag="ht", name="x2")
            if True:
                bprow = bp.tile([1, D], R, tag="kb", bufs=2, name="bprow")
                nc.sync.dma_start(bprow[:], bp_d[None, :].bitcast(R))
                for tb in range(NT):
                    for dt in range(2):
                        pp = psDE.tile([P, 512], F32, tag="pj", bufs=4,
                                       name=f"pp{tb}_{dt}")
                        for gp in range(4):
                            nc.tensor.matmul(
                                pp[:],
                                attnT8[:, 2 * gp:2 * gp + 2, ts(tb, P)],
                                wp8[:, 2 * gp:2 * gp + 2, ds(dt * 512, 512)],
                                start=(gp == 0), stop=False, perf_mode=DRM)
                        # bias + residual folded into the PE accumulation
                        # (psum carries 32x the true values)
                        nc.tensor.matmul(pp[:], onesP32[:],
                                         bprow[:, ds(dt * 512, 512)],
                                         start=False, stop=False)
                        nc.tensor.matmul(pp[:], identR32[:],
                                         xsb[:, tb, ds(dt * 512, 512)],
                                         start=False, stop=True)
                        nc.scalar.activation(x2[:, tb, ds(dt * 512, 512)],
                                             pp[:], AF.Copy,
                                             scale=1.0 / 32.0)

            # ============ Phase E: LN2 (affine folded into W1/b1 on host),
            # transpose into fp8 pair (h28, dh28); stash x2+b2 ==============
            H2x = bp.tile([P, DB, 2, N], F8, tag="se", bufs=1, name="H2x")
            if True:
                layernorm_transpose(x2, H2x, "e", psDE, 1, delta=True)

            # ================= Phase F: FFN (fp8 DR, 3-chain corrected) ====
            psDE_cm.__exit__(None, None, None)
            psF_cm = tc.tile_pool(name="psF", bufs=1, space="PSUM")
            psF = psF_cm.__enter__()
            if True:
                def ffn1_group(nt, ft, fc, pool, ptag, pbufs, w1t):
                    p1 = pool.tile([P, 512], F32, tag=ptag, bufs=pbufs,
                                   name=f"p1_{nt}_{ft}_{fc}")
                    fcs = ds(fc * P, P)
                    n12 = 0
                    for ch in range(3):
                        wsl, hsl = ((0, 0), (1, 0), (0, 1))[ch]
                        for kp in range(DB // 2):
                            nc.tensor.matmul(
                                p1[:],
                                w1t[:, 2 * kp:2 * kp + 2, wsl, fcs],
                                H2x[:, 2 * kp:2 * kp + 2, hsl,
                                    ds(nt * 512, 512)],
                                start=(n12 == 0), stop=(n12 == 11),
                                perf_mode=DRM)
                            n12 += 1
                    bf = ft * 2 + fc
                    if use_lrelu:
                        mz = bp.tile([P, 512], F32, tag="rb", bufs=4,
                                     name=f"mz{nt}_{bf}")
                        nc.scalar.activation(mz[:], p1[:], AF.Prelu,
                                             bias=b1v[:, bf:bf + 1],
                                             scale=1.0 / SW1,
                                             alpha=0.01)
                    else:
                        z = bp.tile([P, 512], F32, tag="qb", bufs=2,
                                    name=f"z{nt}_{bf}")
                        nc.scalar.activation(z[:], p1[:], AF.Identity,
                                             bias=b1v[:, bf:bf + 1],
                                             scale=1.0 / SW1)
                        zs = bp.tile([P, 512], F32, tag="rb", bufs=4,
                                     name=f"zs{nt}_{bf}")
                        nc.vector.tensor_scalar_mul(zs[:], z[:], 0.01)
                        mz = bp.tile([P, 512], F32, tag="rb", bufs=4,
                                     name=f"mz{nt}_{bf}")
                        nc.vector.tensor_tensor(mz[:], z[:], zs[:], OP.max)
                    # quantize on Pool; residual sub on DVE (Pool was the
                    # FFN1 pacing engine)
                    nc.vector.tensor_copy(zx[:, bf, 0, :], mz[:])
                    nc.vector.tensor_tensor(zx[:, bf, 1, :], mz[:],
                                            zx[:, bf, 0, :], OP.subtract)

                for nt in range(NH):
                    zx = bp.tile([P, FO, 2, 512], F8, tag="at",
                                 name=f"zx{nt}")
                    for ft in range(FF // 256):
                        if ft == 0 and nt == 0:
                            w1t = w1pre
                        else:
                            w1t = bp.tile([P, DB, 2, 256], F8, tag="w1s",
                                          bufs=3, name=f"w1_{nt}_{ft}")
                            nc.gpsimd.dma_start(w1t[:],
                                                w1c_d[ft].bitcast(F8))
                        for fc in range(2):
                            ffn1_group(nt, ft, fc, psF, "fp", 8, w1t)
                    pf2 = [psF.tile([P, 512], F32, tag="fp", bufs=8,
                                    name=f"p2_{nt}_{j}") for j in range(8)]
                    NG = FO // 2   # 16 fo-pairs

                    def w2_pair(nt, g):
                        w2t = bp.tile([P, 2, 2, D], F8, tag="w2s", bufs=3,
                                      name=f"w2_{nt}_{g}")
                        if g < NG // 2:
                            nc.gpsimd.dma_start(w2t[:], w2c_d[g].bitcast(F8))
                        else:
                            # dW2 unread in the uncorrected half: skip it
                            nc.gpsimd.dma_start(w2t[:, :, 0, :],
                                                w2c_d[g][:, :, 0, :]
                                                .bitcast(F8))
                        return w2t

                    def ffn2_chains(g, w2t, tb, dt, start):
                        for ch in range(3):
                            if ch == 1 and g >= NG // 2:
                                # dW2 correction on half the f-range is
                                # enough for the error budget
                                continue
                            wsl, zsl = ((0, 0), (1, 0), (0, 1))[ch]
                            nc.tensor.matmul(
                                pf2[tb * 2 + dt][:],
                                zx[:, 2 * g:2 * g + 2, zsl, ts(tb, P)],
                                w2t[:, :, wsl, ds(dt * 512, 512)],
                                start=(start and ch == 0), stop=False,
                                perf_mode=DRM)

                    for g in range(NG - 1):
                        w2t = w2_pair(nt, g)
                        for tb in range(4):
                            for dt in range(2):
                                ffn2_chains(g, w2t, tb, dt, g == 0)
                    # last fo-pair: close each psum group in turn; fold the
                    # residual (64*(x2+b2)) via identity matmul, evac 1/64
                    gl = NG - 1
                    w2t = w2_pair(nt, gl)
                    for tb in range(4):
                        for dt in range(2):
                            ffn2_chains(gl, w2t, tb, dt, False)
                            nc.tensor.matmul(
                                pf2[tb * 2 + dt][:], onesP64[:],
                                b2row[:, ds(dt * 512, 512)],
                                start=False, stop=False)
                            nc.tensor.matmul(
                                pf2[tb * 2 + dt][:], identR64[:],
                                x2[:, nt * 4 + tb, ds(dt * 512, 512)],
                                start=False, stop=True)
                            rows = ds(nt * 512 + tb * P, P)
                            og = bp.tile([P, 512], F32, tag="rb", bufs=4,
                                         name=f"og{nt}_{tb}_{dt}")
                            nc.scalar.activation(og[:], pf2[tb * 2 + dt][:],
                                                 AF.Copy, scale=1.0 / 64.0)
                            nc.sync.dma_start(out_d[rows, ds(dt * 512, 512)],
                                              og[:])
            psF_cm.__exit__(None, None, None)
    nc.compile()
    return nc


def get_nc():
    global _CACHED_NC
    if _CACHED_NC is None:
        _CACHED_NC = build_nc()
    return _CACHED_NC


def _q8pair(w, s):
    """fp8(s*w) and same-scale fp8 residual, as uint8 bit views."""
    hi = (s * w).astype(E4M3)
    lo = (s * w - hi.astype(np.float32)).astype(E4M3)
    return (np.ascontiguousarray(hi).view(np.uint8),
            np.ascontiguousarray(lo).view(np.uint8))


def prep_weights(inputs):
    f32 = lambda k: np.asarray(inputs[k], dtype=np.float64)
    g2, c2 = f32("ln2_g"), f32("ln2_b")
    W1, b1, W2 = f32("W1"), f32("b1"), f32("W2")
    # fold LN2 affine into W1/b1
    W1f = W1 * g2[:, None]
    b1f = (b1 + c2 @ W1).astype(np.float32)
    w1l = W1f.reshape(DB, P, FF).transpose(1, 0, 2).astype(np.float32)
    w2l = W2.reshape(FO, P, D).transpose(1, 0, 2).astype(np.float32)
    W18, dW18 = _q8pair(w1l, SW1)   # [P, DB, FF] uint8 views
    W28, dW28 = _q8pair(w2l, SW2)   # [P, FO, D]
    # pack (W, dW) pairs into per-tile contiguous blobs
    w1s = np.stack([W18, dW18], axis=2)            # [P, DB, 2, FF]
    W1c = np.ascontiguousarray(
        w1s.reshape(P, DB, 2, FF // 256, 256)
        .transpose(3, 0, 1, 2, 4))                 # [16, P, DB, 2, 256]
    w2s_ = np.stack([W28, dW28], axis=2)           # [P, FO, 2, D]
    W2c = np.ascontiguousarray(
        w2s_.reshape(P, FO // 2, 2, 2, D)
        .transpose(1, 0, 2, 3, 4))                 # [16, P, 2, 2, D]
    Wp = f32("Wproj")
    Wp8 = np.ascontiguousarray(
        (SWP * Wp.reshape(EBS, P, D).transpose(1, 0, 2))
        .astype(E4M3)).view(np.uint8)
    g1, c1 = f32("ln1_g"), f32("ln1_b")

    def qkvfold(wname, bname):
        W, b = f32(wname), f32(bname)          # [H, D, HS], [H, HS]
        Wf = W * g1[None, :, None]
        bf = b + np.einsum('d,hds->hs', c1, W)
        wl = (Wf.transpose(1, 0, 2).reshape(DB, P, H * HS)
              .transpose(1, 0, 2).astype(np.float32))
        w8 = np.ascontiguousarray((SWP * wl).astype(E4M3)).view(np.uint8)
        return w8, np.ascontiguousarray((SWP * bf).astype(np.float32))

    Wq8, bq32 = qkvfold("Wq", "bq")
    Wk8, bk32 = qkvfold("Wk", "bk")
    Wv8, bv32 = qkvfold("Wv", "bv")
    w = {k: np.ascontiguousarray(np.asarray(inputs[k], dtype=np.float32))
         for k in ("bproj", "b2")}
    w.update(W1c=W1c, b1f=b1f, W2c=W2c, Wp8=Wp8,
             Wq8=Wq8, bq=bq32, Wk8=Wk8, bk=bk32, Wv8=Wv8, bv=bv32)
    return w


def kernel(**inputs):
    nc = get_nc()
    x = np.ascontiguousarray(np.asarray(inputs["x"], dtype=np.float32))
    B = x.shape[0]
    weights = prep_weights(inputs)
    in_maps = [dict(weights, x=x[b]) for b in range(B)]
    res = run_bass_kernel_spmd(nc, in_maps, list(range(B)))
    return np.stack([res.results[b]["out"] for b in range(B)], axis=0)



# revision 53
# speedup vs baseline: 1.3790x; 1.0189x over previous
"""Trainium2 Bass kernel for a pre-LN transformer block (MHA + FFN).

Data-parallel over batch: 8 NeuronCores, one batch element each.

Speed comes from fp8(e4m3) DoubleRow matmuls (0.5 PE cycles/row over
k-tile pairs = 4x the f32r MAC rate), with precision recovered where it
matters:
  - QKV / attnV / proj run plain fp8 DR (their noise contribution to the
    output is tiny, measured ~5e-3 combined);
  - FFN1/FFN2 run "3-chain corrected" DR:
        y @ W ~= y8@W8 + y8@dW8 + dy8@W8
    where dW8/dy8 are same-scale fp8 residuals (subnormal range), giving
    ~0.1% noise at 0.75x the f32r cycle count. FFN2's dW chain covers
    only half the f-range (error budget allows it).
  - attention scores stay f32r; softmax exp runs on ACT with a -2 bias
    so e^s fits fp8, the shift cancels in the softmax ratio.

Host-side prep (free): LN affines folded into the consumer weights and
biases, weights quantized + packed per-tile-contiguous (W|dW
interleaved), biases pre-scaled by the fp8 weight scales.

On-device structure: LN1 -> fp8 transpose; V/Q/K DR projections with
bias folded into the PE accumulation via ones-row matmuls; per-head-pair
attention (f32r scores, batched exp->fp8, DR attnV with an appended
ones row producing the softmax denominators); DR proj with residual +
bias accumulated on the PE (identity/ones matmuls); LN2 -> fp8 value +
residual pair; FFN with prelu on ACT, z-quantization on Pool/DVE, and
the residual+b2 again folded into the final PE accumulation.
"""
import sys

for _p in ("/opt/trn_rl_repo", "/root/.axon_site/_ro/trn_rl_repo"):
    if _p not in sys.path:
        sys.path.insert(0, _p)

import numpy as np
import ml_dtypes
import concourse.bass as bass
import concourse.tile as tile
from concourse import bacc, mybir
from concourse.bass import ds, ts
from concourse.bass_utils import run_bass_kernel_spmd
from concourse.masks import make_identity

E4M3 = (ml_dtypes.float8_e4m3fn if hasattr(ml_dtypes, "float8_e4m3fn")
        else ml_dtypes.float8_e4m3)
SW1 = 32.0      # host scale for W1 fp8
SWP = 32.0      # host scale for Wproj fp8
SW2 = 64.0      # host scale for W2 fp8

P = 128
N = 1024          # tokens per core (seq len)
D = 1024          # d_emb
H = 16            # heads
HS = 64           # head size
FF = 4096         # ffn hidden
NT = N // P       # 8 token tiles
DB = D // P       # 8 d blocks
EBS = D // P      # 8 e blocks (qkv out features)
NH = 2            # n halves of 512
LN_EPS = 1e-5

F32 = mybir.dt.float32
R = mybir.dt.float32r
F8 = mybir.dt.float8e4
U8 = mybir.dt.uint8
AF = mybir.ActivationFunctionType
OP = mybir.AluOpType
DRM = mybir.MatmulPerfMode.DoubleRow
FO = FF // P      # 32 f-blocks for FFN2 contraction

_CACHED_NC = None


def build_nc(use_lrelu=True):
    nc = bacc.Bacc("TRN2", target_bir_lowering=False, debug=False, num_devices=8)

    x_d = nc.dram_tensor("x", [N, D], F32, kind="ExternalInput").ap()
    wq8_d = nc.dram_tensor("Wq8", [P, DB, H * HS], U8, kind="ExternalInput").ap()
    bq_d = nc.dram_tensor("bq", [H, HS], F32, kind="ExternalInput").ap()
    wk8_d = nc.dram_tensor("Wk8", [P, DB, H * HS], U8, kind="ExternalInput").ap()
    bk_d = nc.dram_tensor("bk", [H, HS], F32, kind="ExternalInput").ap()
    wv8_d = nc.dram_tensor("Wv8", [P, DB, H * HS], U8, kind="ExternalInput").ap()
    bv_d = nc.dram_tensor("bv", [H, HS], F32, kind="ExternalInput").ap()
    wp8_d = nc.dram_tensor("Wp8", [P, EBS, D], mybir.dt.uint8, kind="ExternalInput").ap()
    bp_d = nc.dram_tensor("bproj", [D], F32, kind="ExternalInput").ap()
    w1c_d = nc.dram_tensor("W1c", [FF // 256, P, DB, 2, 256], U8,
                           kind="ExternalInput").ap()
    b1_d = nc.dram_tensor("b1f", [FF], F32, kind="ExternalInput").ap()
    w2c_d = nc.dram_tensor("W2c", [FO // 2, P, 2, 2, D], U8,
                           kind="ExternalInput").ap()
    b2_d = nc.dram_tensor("b2", [D], F32, kind="ExternalInput").ap()
    out_d = nc.dram_tensor("out", [N, D], F32, kind="ExternalOutput").ap()
    x2pb_d = nc.dram_tensor("x2pb_scratch", [P, NT, D], F32).ap()

    with tile.TileContext(nc) as tc:
        with tc.tile_pool(name="cn", bufs=1) as cp, \
             tc.tile_pool(name="big", bufs=1) as bp:
            # ---- constants / bias vectors (persistent, tiny) ----
            ident = cp.tile([P, P], F32)
            make_identity(nc, ident[:])
            ones_f = cp.tile([P, 1], F32)
            nc.vector.memset(ones_f[:], 1.0)
            ones64 = cp.tile([1, HS], R)
            nc.vector.tensor_copy(ones64[:],
                                  ones_f[0:1, :].to_broadcast([1, HS]))
            onesP = cp.tile([1, P], R)
            nc.vector.tensor_copy(onesP[:],
                                  ones_f[0:1, :].to_broadcast([1, P]))
            epsv = cp.tile([P, 1], F32)
            nc.vector.memset(epsv[:], LN_EPS)
            identR = cp.tile([P, P], R)
            nc.vector.tensor_copy(identR[:], ident[:])
            identR64 = cp.tile([P, P], R)
            nc.vector.tensor_scalar_mul(identR64[:], ident[:], 64.0)
            onesP64 = cp.tile([1, P], R)
            nc.vector.tensor_scalar_mul(onesP64[:], onesP[:], 64.0)
            identR32 = cp.tile([P, P], R)
            nc.vector.tensor_scalar_mul(identR32[:], ident[:], 32.0)
            onesP32 = cp.tile([1, P], R)
            nc.vector.tensor_scalar_mul(onesP32[:], onesP[:], 32.0)
            b2row = cp.tile([1, D], R)
            nc.sync.dma_start(b2row[:], b2_d[None, :].bitcast(R))

            # x load first so the big DMA isn't stuck behind the
            # scattered little bias loads
            xsb = bp.tile([P, NT, D], R, tag="at", name="xsb")
            xr3 = x_d.rearrange("(t p) d -> p t d", p=P)
            for tb in range(NT):
                nc.sync.dma_start(xsb[:, tb, :], xr3[:, tb, :].bitcast(R))

            # biases arrive pre-folded (LN affine) and pre-scaled (x32)
            bqv = cp.tile([P, EBS], F32)
            nc.sync.dma_start(bqv[:], bq_d.rearrange("h s -> (h s)")
                              .rearrange("(b p) -> p b", p=P))
            bkv = cp.tile([P, EBS], F32)
            nc.sync.dma_start(bkv[:], bk_d.rearrange("h s -> (h s)")
                              .rearrange("(b p) -> p b", p=P))
            bvrow = cp.tile([1, H * HS], R)
            nc.sync.dma_start(bvrow[:], bv_d.rearrange("h s -> (h s)")
                              [None, :].bitcast(R))
            b1v = cp.tile([P, FF // P], F32)
            nc.sync.dma_start(b1v[:], b1_d.rearrange("(b p) -> p b", p=P))
            bneg2 = cp.tile([P, 1], F32)
            nc.vector.memset(bneg2[:], -2.0)
            wv8t = bp.tile([P, DB, H * HS], F8, tag="wv8", name="wv8t")
            nc.sync.dma_start(wv8t[:], wv8_d.bitcast(F8))
            wq8t = bp.tile([P, DB, H * HS], F8, tag="wq8", name="wq8t")
            nc.sync.dma_start(wq8t[:], wq8_d.bitcast(F8))
            wk8t = bp.tile([P, DB, H * HS], F8, tag="wk8", name="wk8t")
            nc.sync.dma_start(wk8t[:], wk8_d.bitcast(F8))

            # LN stats scratch (reused for LN2 by tag)
            st_sum = cp.tile([P, NT], F32)
            st_sq = cp.tile([P, NT], F32)
            st_var = cp.tile([P, NT], F32)
            st_rs = cp.tile([P, NT], F32)
            st_nm = cp.tile([P, NT], F32)
            st_vh = cp.tile([P, NT], F32)
            st_t = cp.tile([P, NT], F32)
            st_ih = cp.tile([P, NT], mybir.dt.int32)

            def layernorm_transpose(src, dst, pfx, pspool, trbufs,
                                    delta=False):
                """src: [P, NT, D] token layout (f32) -> dst [P, DB, N] fp8
                feature layout (normalized, no affine -- folded into the
                consumer weights on the host). With delta=True, dst is
                [P, DB, 2, N]: slot 0 = fp8 value, slot 1 = fp8 residual.
                Stats run as one batched 8-wide chain over all token tiles.
                """
                def stats_group(g0, g1):
                    for tb in range(g0, g1):
                        nc.vector.reduce_sum(st_sum[:, tb:tb + 1],
                                             src[:, tb, :],
                                             axis=mybir.AxisListType.X)
                        sq = bp.tile([P, D], F32, tag="qb", bufs=2,
                                     name=f"sq{pfx}")
                        nc.scalar.activation(sq[:], src[:, tb, :], AF.Square,
                                             accum_out=st_sq[:, tb:tb + 1])
                    sm = st_sum[:, g0:g1]
                    var = st_var[:, g0:g1]
                    rs = st_rs[:, g0:g1]
                    nm = st_nm[:, g0:g1]
                    ih = st_ih[:, g0:g1]
                    vh = st_vh[:, g0:g1]
                    tt = st_t[:, g0:g1]
                    i32 = mybir.dt.int32
                    # var = sq/D - (sum/D)^2 + eps   (depth-3 chain)
                    nc.vector.tensor_tensor(tt, sm, sm, OP.mult)
                    nc.vector.tensor_scalar(var, tt, -1.0 / (D * D), LN_EPS,
                                            OP.mult, OP.add)
                    nc.vector.tensor_scalar_mul(tt, st_sq[:, g0:g1], 1.0 / D)
                    nc.vector.tensor_tensor(var, tt, var, OP.add)
                    # rstd = rsqrt(var), DVE-only (bit hack + 2 Newton steps)
                    # so the ACT engine never needs the sqrt table set
                    nc.vector.tensor_scalar(ih, var.bitcast(i32), 1, None,
                                            OP.arith_shift_right)
                    nc.vector.tensor_scalar(rs.bitcast(i32), ih, -1,
                                            0x5F3759DF, OP.mult, OP.add)
                    nc.vector.tensor_scalar_mul(vh, var, -0.5)
                    for _ in range(2):
                        nc.vector.tensor_tensor(tt, rs, rs, OP.mult)
                        nc.vector.tensor_tensor(tt, tt, vh, OP.mult)
                        nc.vector.tensor_scalar(tt, tt, 1.0, 1.5,
                                                OP.mult, OP.add)
                        nc.vector.tensor_tensor(rs, rs, tt, OP.mult)
                    # nm = -(sum/D)*rstd
                    nc.vector.tensor_tensor(nm, sm, rs, OP.mult)
                    nc.vector.tensor_scalar_mul(nm, nm, -1.0 / D)

                for (a, b) in ((0, 1), (1, 2), (2, 4), (4, 8)):
                    stats_group(a, b)
                for tb in range(NT):
                    tnorm = bp.tile([P, D], R, tag="kb", bufs=2,
                                    name=f"tn{pfx}")
                    nc.gpsimd.tensor_scalar(tnorm[:], src[:, tb, :],
                                             st_rs[:, tb:tb + 1],
                                             st_nm[:, tb:tb + 1],
                                             OP.mult, OP.add)
                    for dh in range(4):
                        pt4 = pspool.tile([P, 2, 512], R, tag="tr",
                                          bufs=2,
                                          name=f"ptr{pfx}_{tb}_{dh}")
                        for j in range(2):
                            nc.tensor.transpose(pt4[:, j, 0:P],
                                                tnorm[:, ts(dh * 2 + j, P)],
                                                identR[:])
                        d0 = dh * 2
                        if delta:
                            nc.scalar.activation(
                                dst[:, d0:d0 + 2, 0, ts(tb, P)],
                                pt4[:, :, 0:P], AF.Copy)
                            nc.vector.tensor_tensor(
                                dst[:, d0:d0 + 2, 1, ts(tb, P)],
                                pt4[:, :, 0:P],
                                dst[:, d0:d0 + 2, 0, ts(tb, P)],
                                OP.subtract)
                        else:
                            nc.vector.tensor_copy(
                                dst[:, d0:d0 + 2, ts(tb, P)],
                                pt4[:, :, 0:P])

            # ================= Phase A: LN1 + transpose ====================
            HT = bp.tile([P, DB, N], F8, tag="h8", name="HT")
            psAB_cm = tc.tile_pool(name="psAB", bufs=1, space="PSUM")
            psAB = psAB_cm.__enter__()
            layernorm_transpose(xsb, HT, "a", psAB, 1)

            # ================= Phase B0: V projection ======================
            Vaug = bp.tile([P, NT, H, HS + 1], F8, tag="v8", name="Vaug")
            nc.vector.tensor_copy(
                Vaug[:, :, :, HS:HS + 1],
                ones_f[:, None, :].to_broadcast([P, NT, H, 1]))
            if True:
                for eh in range(2):
                    for tb in range(NT):
                        pv = psAB.tile([P, 512], F32, tag="qkv", bufs=4,
                                       name=f"pv{eh}_{tb}")
                        for kp in range(DB // 2):
                            nc.tensor.matmul(
                                pv[:], HT[:, 2 * kp:2 * kp + 2, ts(tb, P)],
                                wv8t[:, 2 * kp:2 * kp + 2,
                                     ds(eh * 512, 512)],
                                start=(kp == 0), stop=False, perf_mode=DRM)
                        # += 32*bv' via ones row (psum carries 32x values)
                        nc.tensor.matmul(pv[:], onesP[:],
                                         bvrow[:, ds(eh * 512, 512)],
                                         start=False, stop=True)
                        nc.scalar.activation(
                            Vaug[:, tb, eh * 8:(eh + 1) * 8, 0:HS],
                            pv[:].rearrange("p (h s) -> p h s", s=HS), AF.Copy,
                            scale=1.0 / 32.0)

            # ============ Phase BC: Q/K per e-block fused with attention ===
            psAB_cm.__exit__(None, None, None)
            psBC_cm = tc.tile_pool(name="psBC", bufs=1, space="PSUM")
            psBC = psBC_cm.__enter__()
            attnT8 = bp.tile([P, EBS, N], F8, tag="a8", name="attnT8")
            # prefetch Wproj (fp8) during attention
            wp8 = bp.tile([P, EBS, D], F8, tag="sh", bufs=1, name="wp8")
            nc.sync.dma_start(wp8[:], wp8_d.bitcast(F8))

            if True:
                for eb in range(EBS):
                    Qb = bp.tile([P, N], R, tag="qb", bufs=2, name=f"Qb{eb}")
                    Kb = bp.tile([P, N], R, tag="kb", bufs=2, name=f"Kb{eb}")
                    ecs = ds(eb * P, P)
                    for nh in range(NH):
                        pq = psBC.tile([P, 512], F32, tag="qk", bufs=2,
                                       name=f"pq{eb}_{nh}")
                        for kp in range(DB // 2):
                            nc.tensor.matmul(pq[:],
                                             wq8t[:, 2 * kp:2 * kp + 2, ecs],
                                             HT[:, 2 * kp:2 * kp + 2,
                                                ds(nh * 512, 512)],
                                             start=(kp == 0), stop=(kp == 3),
                                             perf_mode=DRM)
                        nc.vector.tensor_scalar_add(Qb[:, ds(nh * 512, 512)],
                                                    pq[:], bqv[:, eb:eb + 1])
                        pk = psBC.tile([P, 512], F32, tag="qk", bufs=2,
                                       name=f"pk{eb}_{nh}")
                        for kp in range(DB // 2):
                            nc.tensor.matmul(pk[:],
                                             wk8t[:, 2 * kp:2 * kp + 2, ecs],
                                             HT[:, 2 * kp:2 * kp + 2,
                                                ds(nh * 512, 512)],
                                             start=(kp == 0), stop=(kp == 3),
                                             perf_mode=DRM)
                        nc.vector.tensor_scalar_add(Kb[:, ds(nh * 512, 512)],
                                                    pk[:], bkv[:, eb:eb + 1])

                    # attention for heads 2eb (partitions 0:64) and
                    # 2eb+1 (partitions 64:128), per n-half of 512
                    for nh in range(NH):
                        pts = [bp.tile([P, NT, 512], F8, tag="p8", bufs=2,
                                       name=f"PT{eb}_{nh}_{i}")
                               for i in range(2)]
                        # scoresT[m, n] = sum_s K[m,s] Q[n,s]; exp via ACT
                        # (Qb/Kb carry 32x values -> scale 0.125/1024); the
                        # -2 bias keeps exp within fp8 range, cancels in the
                        # softmax ratio
                        for mq in range(NT // 2):
                            for i in range(2):
                                base = i * HS
                                pss = psBC.tile([P, 2, 512], F32, tag="sc",
                                                bufs=2,
                                                name=f"ps{eb}{nh}{mq}{i}")
                                for mj in range(2):
                                    nc.tensor.matmul(
                                        pss[:, mj, :],
                                        Kb[base:base + HS,
                                           ts(mq * 2 + mj, P)],
                                        Qb[base:base + HS,
                                           ds(nh * 512, 512)],
                                        start=True, stop=True)
                                nc.scalar.activation(
                                    pts[i][:, 2 * mq:2 * mq + 2, :], pss[:],
                                    AF.Exp, scale=0.125 / 1024.0,
                                    bias=bneg2[:])
                        pas = [psBC.tile([HS + 1, 512], F32, tag="at65",
                                         bufs=2, name=f"pa{eb}_{nh}_{i}")
                               for i in range(2)]
                        for mp in range(NT // 2):
                            for i in range(2):
                                nc.tensor.matmul(pas[i][:],
                                                 Vaug[:, 2 * mp:2 * mp + 2,
                                                      2 * eb + i, :],
                                                 pts[i][:, 2 * mp:2 * mp + 2,
                                                        :],
                                                 start=(mp == 0),
                                                 stop=(mp == NT // 2 - 1),
                                                 perf_mode=DRM)
                        for i in range(2):
                            base = i * HS
                            rec = bp.tile([1, 512], F32, tag="rb", bufs=4,
                                          name=f"rc{eb}_{nh}_{i}")
                            nc.vector.reciprocal(rec[:],
                                                 pas[i][HS:HS + 1, :])
                            rbs = bp.tile([HS, 512], F32, tag="rb", bufs=4,
                                          name=f"rb{eb}_{nh}_{i}")
                            nc.gpsimd.partition_broadcast(rbs[:], rec[:])
                            dstA = attnT8[base:base + HS, eb,
                                          ds(nh * 512, 512)]
                            nc.vector.tensor_tensor(dstA, pas[i][0:HS, :],
                                                    rbs[:], OP.mult)

            # w1(ft0) prefetch into "se" (frees at end of attention);
            # high priority so the DMA issues as soon as the slot frees
            w1pre = bp.tile([P, DB, 2, 256], F8, tag="w1s", bufs=3,
                            name="w1pre")
            with tc.high_priority():
                nc.gpsimd.dma_start(w1pre[:], w1c_d[0].bitcast(F8))

            # ================= Phase D: proj + residual ====================
            psBC_cm.__exit__(None, None, None)
            psDE_cm = tc.tile_pool(name="psDE", bufs=1, space="PSUM")
            psDE = psDE_cm.__enter__()
            x2 = bp.tile([P, NT, D], R, tag="ht", name="x2")
            if True:
                bprow = bp.tile([1, D], R, tag="kb", bufs=2, name="bprow")
                nc.sync.dma_start(bprow[:], bp_d[None, :].bitcast(R))
                for tb in range(NT):
                    for dt in range(2):
                        pp = psDE.tile([P, 512], F32, tag="pj", bufs=4,
                                       name=f"pp{tb}_{dt}")
                        for gp in range(4):
                            nc.tensor.matmul(
                                pp[:],
                                attnT8[:, 2 * gp:2 * gp + 2, ts(tb, P)],
                                wp8[:, 2 * gp:2 * gp + 2, ds(dt * 512, 512)],
                                start=(gp == 0), stop=False, perf_mode=DRM)
                        # bias + residual folded into the PE accumulation
                        # (psum carries 32x the true values)
                        nc.tensor.matmul(pp[:], onesP32[:],
                                         bprow[:, ds(dt * 512, 512)],
                                         start=False, stop=False)
                        nc.tensor.matmul(pp[:], identR32[:],
                                         xsb[:, tb, ds(dt * 512, 512)],
                                         start=False, stop=True)
                        nc.scalar.activation(x2[:, tb, ds(dt * 512, 512)],
                                             pp[:], AF.Copy,
                                             scale=1.0 / 32.0)

            # ============ Phase E: LN2 (affine folded into W1/b1 on host),
            # transpose into fp8 pair (h28, dh28); stash x2+b2 ==============
            H2x = bp.tile([P, DB, 2, N], F8, tag="se", bufs=1, name="H2x")
            if True:
                layernorm_transpose(x2, H2x, "e", psDE, 1, delta=True)

            # ================= Phase F: FFN (fp8 DR, 3-chain corrected) ====
            psDE_cm.__exit__(None, None, None)
            psF_cm = tc.tile_pool(name="psF", bufs=1, space="PSUM")
            psF = psF_cm.__enter__()
            if True:
                def ffn1_group(nt, ft, fc, pool, ptag, pbufs, w1t):
                    p1 = pool.tile([P, 512], F32, tag=ptag, bufs=pbufs,
                                   name=f"p1_{nt}_{ft}_{fc}")
                    fcs = ds(fc * P, P)
                    # dW1 correction on 3/4 of the k-range is enough for
                    # the error budget (skip chain B's last k-pair)
                    steps = [(ch, kp) for ch in range(3)
                             for kp in range(DB // 2)
                             if not (ch == 1 and kp == 3)]
                    for n12, (ch, kp) in enumerate(steps):
                        wsl, hsl = ((0, 0), (1, 0), (0, 1))[ch]
                        nc.tensor.matmul(
                            p1[:],
                            w1t[:, 2 * kp:2 * kp + 2, wsl, fcs],
                            H2x[:, 2 * kp:2 * kp + 2, hsl,
                                ds(nt * 512, 512)],
                            start=(n12 == 0), stop=(n12 == len(steps) - 1),
                            perf_mode=DRM)
                    bf = ft * 2 + fc
                    if use_lrelu:
                        mz = bp.tile([P, 512], F32, tag="rb", bufs=4,
                                     name=f"mz{nt}_{bf}")
                        nc.scalar.activation(mz[:], p1[:], AF.Prelu,
                                             bias=b1v[:, bf:bf + 1],
                                             scale=1.0 / SW1,
                                             alpha=0.01)
                    else:
                        z = bp.tile([P, 512], F32, tag="qb", bufs=2,
                                    name=f"z{nt}_{bf}")
                        nc.scalar.activation(z[:], p1[:], AF.Identity,
                                             bias=b1v[:, bf:bf + 1],
                                             scale=1.0 / SW1)
                        zs = bp.tile([P, 512], F32, tag="rb", bufs=4,
                                     name=f"zs{nt}_{bf}")
                        nc.vector.tensor_scalar_mul(zs[:], z[:], 0.01)
                        mz = bp.tile([P, 512], F32, tag="rb", bufs=4,
                                     name=f"mz{nt}_{bf}")
                        nc.vector.tensor_tensor(mz[:], z[:], zs[:], OP.max)
                    # quantize on Pool; residual sub on DVE (Pool was the
                    # FFN1 pacing engine)
                    nc.vector.tensor_copy(zx[:, bf, 0, :], mz[:])
                    nc.vector.tensor_tensor(zx[:, bf, 1, :], mz[:],
                                            zx[:, bf, 0, :], OP.subtract)

                for nt in range(NH):
                    zx = bp.tile([P, FO, 2, 512], F8, tag="at",
                                 name=f"zx{nt}")
                    for ft in range(FF // 256):
                        if ft == 0 and nt == 0:
                            w1t = w1pre
                        else:
                            w1t = bp.tile([P, DB, 2, 256], F8, tag="w1s",
                                          bufs=3, name=f"w1_{nt}_{ft}")
                            nc.gpsimd.dma_start(w1t[:],
                                                w1c_d[ft].bitcast(F8))
                        for fc in range(2):
                            ffn1_group(nt, ft, fc, psF, "fp", 8, w1t)
                    pf2 = [psF.tile([P, 512], F32, tag="fp", bufs=8,
                                    name=f"p2_{nt}_{j}") for j in range(8)]
                    NG = FO // 2   # 16 fo-pairs

                    def w2_pair(nt, g):
                        w2t = bp.tile([P, 2, 2, D], F8, tag="w2s", bufs=3,
                                      name=f"w2_{nt}_{g}")
                        if g < NG // 2:
                            nc.gpsimd.dma_start(w2t[:], w2c_d[g].bitcast(F8))
                        else:
                            # dW2 unread in the uncorrected half: skip it
                            nc.gpsimd.dma_start(w2t[:, :, 0, :],
                                                w2c_d[g][:, :, 0, :]
                                                .bitcast(F8))
                        return w2t

                    def ffn2_chains(g, w2t, tb, dt, start):
                        for ch in range(3):
                            if ch == 1 and g >= NG // 2:
                                # dW2 correction on half the f-range is
                                # enough for the error budget
                                continue
                            wsl, zsl = ((0, 0), (1, 0), (0, 1))[ch]
                            nc.tensor.matmul(
                                pf2[tb * 2 + dt][:],
                                zx[:, 2 * g:2 * g + 2, zsl, ts(tb, P)],
                                w2t[:, :, wsl, ds(dt * 512, 512)],
                                start=(start and ch == 0), stop=False,
                                perf_mode=DRM)

                    for g in range(NG - 1):
                        w2t = w2_pair(nt, g)
                        for tb in range(4):
                            for dt in range(2):
                                ffn2_chains(g, w2t, tb, dt, g == 0)
                    # last fo-pair: close each psum group in turn; fold the
                    # residual (64*(x2+b2)) via identity matmul, evac 1/64
                    gl = NG - 1
                    w2t = w2_pair(nt, gl)
                    for tb in range(4):
                        for dt in range(2):
                            ffn2_chains(gl, w2t, tb, dt, False)
                            nc.tensor.matmul(
                                pf2[tb * 2 + dt][:], onesP64[:],
                                b2row[:, ds(dt * 512, 512)],
                                start=False, stop=False)
                            nc.tensor.matmul(
                                pf2[tb * 2 + dt][:], identR64[:],
                                x2[:, nt * 4 + tb, ds(dt * 512, 512)],
                                start=False, stop=True)
                            rows = ds(nt * 512 + tb * P, P)
                            og = bp.tile([P, 512], F32, tag="rb", bufs=4,
                                         name=f"og{nt}_{tb}_{dt}")
                            nc.scalar.activation(og[:], pf2[tb * 2 + dt][:],
                                                 AF.Copy, scale=1.0 / 64.0)
                            nc.sync.dma_start(out_d[rows, ds(dt * 512, 512)],
                                              og[:])
            psF_cm.__exit__(None, None, None)
    nc.compile()
    return nc


def get_nc():
    global _CACHED_NC
    if _CACHED_NC is None:
        _CACHED_NC = build_nc()
    return _CACHED_NC


def _q8pair(w, s):
    """fp8(s*w) and same-scale fp8 residual, as uint8 bit views."""
    hi = (s * w).astype(E4M3)
    lo = (s * w - hi.astype(np.float32)).astype(E4M3)
    return (np.ascontiguousarray(hi).view(np.uint8),
            np.ascontiguousarray(lo).view(np.uint8))


def prep_weights(inputs):
    f32 = lambda k: np.asarray(inputs[k], dtype=np.float64)
    g2, c2 = f32("ln2_g"), f32("ln2_b")
    W1, b1, W2 = f32("W1"), f32("b1"), f32("W2")
    # fold LN2 affine into W1/b1
    W1f = W1 * g2[:, None]
    b1f = (b1 + c2 @ W1).astype(np.float32)
    w1l = W1f.reshape(DB, P, FF).transpose(1, 0, 2).astype(np.float32)
    w2l = W2.reshape(FO, P, D).transpose(1, 0, 2).astype(np.float32)
    W18, dW18 = _q8pair(w1l, SW1)   # [P, DB, FF] uint8 views
    W28, dW28 = _q8pair(w2l, SW2)   # [P, FO, D]
    # pack (W, dW) pairs into per-tile contiguous blobs
    w1s = np.stack([W18, dW18], axis=2)            # [P, DB, 2, FF]
    W1c = np.ascontiguousarray(
        w1s.reshape(P, DB, 2, FF // 256, 256)
        .transpose(3, 0, 1, 2, 4))                 # [16, P, DB, 2, 256]
    w2s_ = np.stack([W28, dW28], axis=2)           # [P, FO, 2, D]
    W2c = np.ascontiguousarray(
        w2s_.reshape(P, FO // 2, 2, 2, D)
        .transpose(1, 0, 2, 3, 4))                 # [16, P, 2, 2, D]
    Wp = f32("Wproj")
    Wp8 = np.ascontiguousarray(
        (SWP * Wp.reshape(EBS, P, D).transpose(1, 0, 2))
        .astype(E4M3)).view(np.uint8)
    g1, c1 = f32("ln1_g"), f32("ln1_b")

    def qkvfold(wname, bname):
        W, b = f32(wname), f32(bname)          # [H, D, HS], [H, HS]
        Wf = W * g1[None, :, None]
        bf = b + np.einsum('d,hds->hs', c1, W)
        wl = (Wf.transpose(1, 0, 2).reshape(DB, P, H * HS)
              .transpose(1, 0, 2).astype(np.float32))
        w8 = np.ascontiguousarray((SWP * wl).astype(E4M3)).view(np.uint8)
        return w8, np.ascontiguousarray((SWP * bf).astype(np.float32))

    Wq8, bq32 = qkvfold("Wq", "bq")
    Wk8, bk32 = qkvfold("Wk", "bk")
    Wv8, bv32 = qkvfold("Wv", "bv")
    w = {k: np.ascontiguousarray(np.asarray(inputs[k], dtype=np.float32))
         for k in ("bproj", "b2")}
    w.update(W1c=W1c, b1f=b1f, W2c=W2c, Wp8=Wp8,
             Wq8=Wq8, bq=bq32, Wk8=Wk8, bk=bk32, Wv8=Wv8, bv=bv32)
    return w


def kernel(**inputs):
    nc = get_nc()
    x = np.ascontiguousarray(np.asarray(inputs["x"], dtype=np.float32))
    B = x.shape[0]
    weights = prep_weights(inputs)
    in_maps = [dict(weights, x=x[b]) for b in range(B)]
    res = run_bass_kernel_spmd(nc, in_maps, list(range(B)))
    return np.stack([res.results[b]["out"] for b in range(B)], axis=0)

